# revision 13
# baseline (speedup 1.0000x reference)
"""Trainium2 Bass kernel for nn_Attention_14955076125505.

Windowed self-attention with relative-position bias:
  x:(8,512,32,32) -> qkv -> 16-head attention(N=1024, d=32) + bias_table[rel_index]
  -> out proj -> (8,512,32,32)

Sharding (8 NeuronCores), v4:
  - tensor-parallel over heads: core c owns heads (2c, 2c+1) for qkv + attention.
  - bias: rel_index from the reference is the deterministic 2D relative-
    position (Toeplitz) layout: idx(q,k) = 63*(qy-ky+31) + (qx-kx+31).
    kernel() verifies this host-side; when it holds, each per-(kb) bias tile
    c[key=128kb+p, head e, q] = exp(table[idx, h]) is a strided window read
    over the 63x63 exp'd table, fetched by ONE DMA per kb with a
    hand-built access pattern (negative strides on the key dims).  This
    replaces the previous GpSimd ap_gather + fp8 + AllToAll frontend
    (~300us/rep).  If rel_index ever fails the check, a gather-based
    fallback graph (v3) is used instead.
  - attention output is shipped UNNORMALIZED (+ the per-(head,q) softmax
    denominator as a 33rd row) through an AllToAll that re-shards by
    batch; the batch owner computes reciprocals, broadcasts them over d via a
    small matmul, normalizes, and runs the output projection for its batch.
Compute dtype bf16 on the TensorEngine (f32 PSUM accumulation), exp on ScalarE,
bias multiply on VectorE.
"""

import sys

if "/opt/trn_rl_repo" not in sys.path:
    sys.path.insert(0, "/opt/trn_rl_repo")

import numpy as np
import ml_dtypes

B = 8
C = 512
N = 1024  # H*W
HEADS = 16
D = 32
OUP = 512
TABLE = 3969
NCORES = 8
HPC = 2  # heads per core
KCH = 128  # keys gathered per core (fallback path)
SCALE = D ** -0.5

BF = ml_dtypes.bfloat16

_GRAPH_CACHE = {}

# owner-side reciprocal broadcast selectors, one per kc pair of source cores:
# row r of ao_sb[:, kc, :] is (core 2kc + r//64, head-in-core (r%64)//32, d),
# i.e. global head 2*(2kc + r//64) + (r%64)//32; E16[kc][head, r] = 1 selects
# that head's reciprocal row.
E16 = np.zeros((16, 4, 128), np.float32)
for _kc in range(4):
    for _r in range(128):
        _head = 2 * (2 * _kc + _r // 64) + (_r % 64) // 32
        E16[_head, _kc, _r] = 1.0


def _toeplitz_rel_index():
    yy, xx = np.meshgrid(np.arange(32), np.arange(32), indexing="ij")
    coords = np.stack([yy.ravel(), xx.ravel()])
    rel = coords[:, :, None] - coords[:, None, :]
    rel[0] += 31
    rel[1] += 31
    rel[0] *= 63
    return rel.sum(0).ravel()  # (N*N,) q-major


# ============================================================================
# v4 graph: DMA-window bias (Toeplitz fast path)
# ============================================================================

def _build_graph(repeat=1, collectives=True, num_devices=NCORES, skip_bias=False,
                 skip_mult=False, at_bufs=4, st_bufs=1):
    import concourse.bass as bass
    import concourse.mybir as mybir
    import concourse.tile as tile
    from concourse import bacc

    fp32 = mybir.dt.float32
    bf16 = mybir.dt.bfloat16

    nc = bacc.Bacc(
        "TRN2",
        target_bir_lowering=False,
        debug=False,
        enable_asserts=True,
        num_devices=num_devices,
    )

    # ---- kernel I/O (per-core shards, prepared host-side) ----
    x_d = nc.dram_tensor("x", [B, C, N], bf16, kind="ExternalInput").ap()
    wqk_d = nc.dram_tensor("w_qkT", [C, 128], bf16, kind="ExternalInput").ap()
    wv_d = nc.dram_tensor("w_vT", [C, 2 * D], bf16, kind="ExternalInput").ap()
    wo_d = nc.dram_tensor("w_outT", [OUP, OUP], bf16, kind="ExternalInput").ap()
    bo_d = nc.dram_tensor("b_out4", [128, 4], fp32, kind="ExternalInput").ap()
    e16_d = nc.dram_tensor("e16", [16, 4, 128], bf16, kind="ExternalInput").ap()
    # exp'd bias table for this core's 2 heads, window-expanded and
    # replicated 4x over ky (see _prepare_in_maps):
    # bias_e[e, 64512*ky + 2016*kx + 32*u + qx] = exp(table[63*u + qx +
    # 31 - kx, 2c+e]); per (kb, e) the c tile is then a 3-dim all-positive
    # strided DMA with 1024-element contiguous runs.
    be_d = nc.dram_tensor("bias_e", [2, 258048], bf16, kind="ExternalInput").ap()
    out_d = nc.dram_tensor("out", [OUP, N], fp32, kind="ExternalOutput").ap()

    # attention-out a2a: 66 rows = 2 heads x (32 out + 1 denominator)
    ao_in = nc.dram_tensor("ao_a2a_in", [NCORES, 2 * (D + 1), N], bf16).ap()
    ao_out = nc.dram_tensor("ao_a2a_out", [NCORES, 2 * (D + 1), N], bf16).ap()

    RG = [list(range(NCORES))]

    def bias_window_ap(kb, e):
        """Source AP over bias_e for the c tile of key-block kb, head e:
        element (p=(ky',kx), qy, qx) reads the exp'd bias for
        key=(4*kb+ky', kx), query=(qy, qx): addr = e*258048 + 64512*ky' +
        2016*kx + 32*(qy + 31 - 4*kb - ky') + qx, i.e. ky' stride
        64512-32 = 64480 against the ky-replicated table."""
        return bass.AP(
            tensor=be_d.tensor,
            offset=e * 258048 + (31 - 4 * kb) * 32,
            ap=[[64480, 4], [2016, 32], [1, 1024]],
        )

    with tile.TileContext(nc) as tc:
        with (
            tc.tile_pool(name="const", bufs=1) as cp,
            tc.tile_pool(name="persist", bufs=1) as pp,
            tc.tile_pool(name="work", bufs=3) as wp,
            tc.tile_pool(name="psum", bufs=2, space="PSUM") as psp,
            tc.tile_pool(name="psav", bufs=2, space="PSUM") as pav,
        ):
            # ================= constants =================
            wqk_sb = cp.tile([128, 4, 128], bf16)
            nc.sync.dma_start(wqk_sb[:, :, :], wqk_d.rearrange("(kc p) m -> p kc m", p=128))
            wv_sb = cp.tile([128, 4, 2 * D], bf16)
            nc.sync.dma_start(wv_sb[:, :, :], wv_d.rearrange("(kc p) m -> p kc m", p=128))
            wo_sb = cp.tile([128, 4, OUP], bf16)
            nc.sync.dma_start(wo_sb[:, :, :], wo_d.rearrange("(kc p) m -> p kc m", p=128))
            bo_sb = cp.tile([128, 4], fp32)
            nc.sync.dma_start(bo_sb[:, :], bo_d)
            e16_sb = cp.tile([16, 4, 128], bf16)
            nc.sync.dma_start(e16_sb[:, :, :], e16_d)

            for _rep in range(repeat):
                # bias tiles for all 8 key-blocks (shared across batches)
                c_sb = pp.tile([128, 8, HPC * N], bf16, name="c_sb")
                if skip_bias:
                    nc.vector.memset(c_sb[:, :, :], 1.0)
                else:
                    for kb in range(8):
                        for e in range(HPC):
                            nc.sync.dma_start(c_sb[:, kb, e * N:(e + 1) * N],
                                              bias_window_ap(kb, e))

                # ================= qkv projection (all 8 batches) =================
                q_sb = pp.tile([64, B, N], bf16, name="q_sb")
                k_sb = pp.tile([64, B, N], bf16, name="k_sb")
                v_sb = pp.tile([128, B, 8, HPC, D + 1], bf16, name="v_sb")
                nc.vector.memset(v_sb[:, :, :, :, D], 1.0)

                for b in range(B):
                    x_t = wp.tile([128, 4, N], bf16, tag="xt", bufs=2)
                    nc.sync.dma_start(x_t[:, :, :], x_d[b].rearrange("(kc p) n -> p kc n", p=128))

                    qk_ps = psp.tile([128, N], fp32, tag="st", bufs=st_bufs,
                                     padded_shape=[128, 2 * N])
                    for kc in range(4):
                        for half in range(2):
                            sl = slice(512 * half, 512 * (half + 1))
                            nc.tensor.matmul(
                                qk_ps[:, sl],
                                wqk_sb[:, kc, :],
                                x_t[:, kc, sl],
                                start=(kc == 0),
                                stop=(kc == 3),
                            )
                    nc.vector.tensor_copy(q_sb[:, b, :], qk_ps[0:64, :])
                    nc.vector.tensor_copy(k_sb[:, b, :], qk_ps[64:128, :])

                    for tbq in range(2):
                        v_ps = psp.tile([128, 4, 2 * D], fp32, tag="st", bufs=st_bufs,
                                        padded_shape=[128, 4, 2 * N // 4])
                        for tb4 in range(4):
                            tb = 4 * tbq + tb4
                            for kc in range(4):
                                nc.tensor.matmul(
                                    v_ps[:, tb4, :],
                                    x_t[:, kc, 128 * tb:128 * (tb + 1)],
                                    wv_sb[:, kc, :],
                                    start=(kc == 0),
                                    stop=(kc == 3),
                                )
                        nc.vector.tensor_copy(
                            v_sb[:, b, 4 * tbq:4 * (tbq + 1), :, 0:D],
                            v_ps[:, :, :].rearrange("p t (h d) -> p t h d", h=HPC),
                        )

                # ================= attention =================
                # attnout_sb rows 64h+d: unnormalized out (d<32) + denominator
                # (d=32); head blocks start at partitions 0/64 (32-aligned)
                attnout_sb = pp.tile([128, B, N], bf16, name="attnout_sb")

                for b in range(B):
                    avh = [pav.tile([D + 1, N], fp32, tag=f"av{h}", bufs=1,
                                    padded_shape=[128, N], name=f"av{h}")
                           for h in range(HPC)]
                    for kb in range(8):
                        st2 = psp.tile([128, 2 * N], fp32, tag="st", bufs=st_bufs)
                        for h in range(HPC):
                            for half in range(2):
                                sl = slice(512 * half, 512 * (half + 1))
                                nc.tensor.matmul(
                                    st2[:, h * N + 512 * half:h * N + 512 * (half + 1)],
                                    k_sb[32 * h:32 * (h + 1), b, 128 * kb:128 * (kb + 1)],
                                    q_sb[32 * h:32 * (h + 1), b, sl],
                                    start=True,
                                    stop=True,
                                )
                        at = wp.tile([128, 2 * N], bf16, tag="attn", bufs=at_bufs)
                        nc.scalar.activation(at[:, :], st2[:, :], mybir.ActivationFunctionType.Exp)
                        if not skip_mult:
                            nc.vector.tensor_mul(at[:, :], at[:, :], c_sb[:, kb, :])
                        for h in range(HPC):
                            for half in range(2):
                                sl = slice(512 * half, 512 * (half + 1))
                                nc.tensor.matmul(
                                    avh[h][:, sl],
                                    v_sb[:, b, kb, h, :],
                                    at[:, h * N + 512 * half:h * N + 512 * (half + 1)],
                                    start=(kb == 0),
                                    stop=(kb == 7),
                                )
                    for h in range(HPC):
                        nc.vector.tensor_copy(
                            attnout_sb[64 * h:64 * h + D + 1, b, :],
                            avh[h][:, :],
                        )
                    nc.sync.dma_start(ao_in[b][0:D + 1], attnout_sb[0:D + 1, b, :])
                    nc.sync.dma_start(ao_in[b][D + 1:], attnout_sb[64:64 + D + 1, b, :])

                # ================= all-to-all: heads -> batch =================
                if collectives:
                    nc.gpsimd.collective_compute(
                        "AllToAll",
                        mybir.AluOpType.bypass,
                        replica_groups=RG,
                        ins=[ao_in.opt()],
                        outs=[ao_out.opt()],
                    )
                else:
                    nc.gpsimd.dma_start(ao_out.opt(), ao_in.opt())

                # ================= normalize + output projection (my batch) =======
                den_sb = wp.tile([16, N], bf16, tag="den", bufs=1)
                nc.sync.dma_start(
                    den_sb[:, :],
                    ao_out.rearrange("s (h x) n -> (s h) x n", x=D + 1)[:, D, :],
                )
                rec_sb = wp.tile([16, N], bf16, tag="rec", bufs=1)
                with nc.allow_low_precision(reason="bf16 softmax denominators are within tolerance"):
                    nc.vector.reciprocal(rec_sb[:, :], den_sb[:, :])

                ao_sb = pp.tile([128, 4, N], bf16, name="ao_sb")
                for kc in range(4):
                    for j in range(2):
                        for h in range(HPC):
                            nc.sync.dma_start(
                                ao_sb[64 * j + 32 * h:64 * j + 32 * h + D, kc, :],
                                ao_out[2 * kc + j, (D + 1) * h:(D + 1) * h + D, :],
                            )
                for kc in range(4):
                    bc_ps = pav.tile([128, N], fp32, tag="av0", bufs=1)
                    for half in range(2):
                        sl = slice(512 * half, 512 * (half + 1))
                        nc.tensor.matmul(bc_ps[:, sl], e16_sb[:, kc, :], rec_sb[:, sl],
                                         start=True, stop=True)
                    nc.vector.tensor_mul(ao_sb[:, kc, :], ao_sb[:, kc, :], bc_ps[:, :])

                for mb in range(4):
                    o_ps = psp.tile([128, N], fp32, tag="st", bufs=st_bufs,
                                    padded_shape=[128, 2 * N])
                    for kc in range(4):
                        for half in range(2):
                            sl = slice(512 * half, 512 * (half + 1))
                            nc.tensor.matmul(
                                o_ps[:, sl],
                                wo_sb[:, kc, 128 * mb:128 * (mb + 1)],
                                ao_sb[:, kc, sl],
                                start=(kc == 0),
                                stop=(kc == 3),
                            )
                    o_sb = wp.tile([128, N], fp32, tag="osb", bufs=2)
                    nc.vector.tensor_scalar_add(o_sb[:, :], o_ps[:, :], bo_sb[:, mb:mb + 1])
                    nc.sync.dma_start(out_d[128 * mb:128 * (mb + 1), :], o_sb[:, :])

    nc.compile()
    return nc


# ============================================================================
# v3 fallback graph: GpSimd gather + fp8 + AllToAll bias frontend
# ============================================================================

def _build_graph_gather(repeat=1, collectives=True, num_devices=NCORES, skip_bias=False,
                        skip_mult=False, fp8_bias=True, at_bufs=4, st_bufs=1,
                        split_st=False, skip_gather=False, nchunk=4):
    import concourse.bass as bass
    import concourse.mybir as mybir
    import concourse.tile as tile
    from concourse import bacc

    fp32 = mybir.dt.float32
    bf16 = mybir.dt.bfloat16
    f8 = mybir.dt.float8e4
    i16 = mybir.dt.int16
    bias_dt = f8 if fp8_bias else bf16
    stb = 2 if split_st else st_bufs
    stpad = [128, N if split_st else 2 * N]

    nc = bacc.Bacc(
        "TRN2",
        target_bir_lowering=False,
        debug=False,
        enable_asserts=True,
        num_devices=num_devices,
    )

    x_d = nc.dram_tensor("x", [B, C, N], bf16, kind="ExternalInput").ap()
    wqk_d = nc.dram_tensor("w_qkT", [C, 128], bf16, kind="ExternalInput").ap()
    wv_d = nc.dram_tensor("w_vT", [C, 2 * D], bf16, kind="ExternalInput").ap()
    wo_d = nc.dram_tensor("w_outT", [OUP, OUP], bf16, kind="ExternalInput").ap()
    bo_d = nc.dram_tensor("b_out4", [128, 4], fp32, kind="ExternalInput").ap()
    tab_d = nc.dram_tensor("table", [128, TABLE], fp32, kind="ExternalInput").ap()
    idx_d = nc.dram_tensor("idx", [128, N], i16, kind="ExternalInput").ap()
    e16_d = nc.dram_tensor("e16", [16, 4, 128], bf16, kind="ExternalInput").ap()
    out_d = nc.dram_tensor("out", [OUP, N], fp32, kind="ExternalOutput").ap()

    ebc_in = nc.dram_tensor("ebc_a2a_in", [16, 8, 16384], bias_dt).ap()
    ebc_out = nc.dram_tensor("ebc_a2a_out", [NCORES, HPC, 8, 16384], bias_dt).ap()
    ao_in = nc.dram_tensor("ao_a2a_in", [NCORES, 2 * (D + 1), N], bf16).ap()
    ao_out = nc.dram_tensor("ao_a2a_out", [NCORES, 2 * (D + 1), N], bf16).ap()

    RG = [list(range(NCORES))]

    with tile.TileContext(nc) as tc:
        with (
            tc.tile_pool(name="const", bufs=1) as cp,
            tc.tile_pool(name="persist", bufs=1) as pp,
            tc.tile_pool(name="work", bufs=3) as wp,
            tc.tile_pool(name="gather", bufs=1) as gp,
            tc.tile_pool(name="psum", bufs=2, space="PSUM") as psp,
            tc.tile_pool(name="psav", bufs=2, space="PSUM") as pav,
        ):
            tab_sb = cp.tile([128, TABLE], fp32)
            nc.sync.dma_start(tab_sb[:, :], tab_d)
            idx_sb = cp.tile([128, N], i16)
            nc.sync.dma_start(idx_sb[:, :], idx_d)
            wqk_sb = cp.tile([128, 4, 128], bf16)
            nc.sync.dma_start(wqk_sb[:, :, :], wqk_d.rearrange("(kc p) m -> p kc m", p=128))
            wv_sb = cp.tile([128, 4, 2 * D], bf16)
            nc.sync.dma_start(wv_sb[:, :, :], wv_d.rearrange("(kc p) m -> p kc m", p=128))
            wo_sb = cp.tile([128, 4, OUP], bf16)
            nc.sync.dma_start(wo_sb[:, :, :], wo_d.rearrange("(kc p) m -> p kc m", p=128))
            bo_sb = cp.tile([128, 4], fp32)
            nc.sync.dma_start(bo_sb[:, :], bo_d)
            e16_sb = cp.tile([16, 4, 128], bf16)
            nc.sync.dma_start(e16_sb[:, :, :], e16_d)

            b8full = cp.tile([128, 8, HPC, N], bias_dt)

            NIDX = 16 * N
            NCHUNK = nchunk
            CH = NIDX // NCHUNK

            def emit_frontend():
                for ch in range(NCHUNK):
                    gath = gp.tile([128, CH], fp32, tag="gath", bufs=1)
                    if skip_gather:
                        nc.gpsimd.memset(gath[:, :], 0.01)
                    else:
                        nc.gpsimd.ap_gather(
                            out_ap=gath[:, :],
                            in_ap=tab_sb[:, :],
                            idxs_ap=idx_sb[:, ch * (CH // 16):(ch + 1) * (CH // 16)],
                            channels=128,
                            num_elems=TABLE,
                            d=1,
                            num_idxs=CH,
                        )
                    b8 = gp.tile([128, CH], bias_dt, tag="b8", bufs=1)
                    nc.gpsimd.tensor_copy(b8[:, :], gath[:, :])
                    nc.gpsimd.dma_start(
                        ebc_in.rearrange("r g (ch f) -> g r ch f", f=CH)[:, :, ch, :],
                        b8[:, :],
                    )
                if collectives:
                    nc.gpsimd.collective_compute(
                        "AllToAll",
                        mybir.AluOpType.bypass,
                        replica_groups=RG,
                        ins=[ebc_in.opt()],
                        outs=[ebc_out.opt()],
                    )
                else:
                    nc.gpsimd.dma_start(
                        ebc_out.rearrange("s e g f -> (s e) g f"), ebc_in.opt())

            def emit_loads():
                for kb in range(8):
                    for e in range(HPC):
                        nc.gpsimd.dma_start(
                            b8full[:, kb, e, :],
                            ebc_out[kb, e].rearrange("g (fq q) -> (g fq) q", q=N),
                        )

            for _rep in range(repeat):
                c_sb = pp.tile([128, 8, HPC * N], bf16, name="c_sb")
                if skip_bias:
                    nc.vector.memset(c_sb[:, :, :], 1.0)
                else:
                    if _rep == 0:
                        emit_frontend()
                        emit_loads()
                    for kb in range(8):
                        for e in range(HPC):
                            nc.vector.tensor_scalar_add(
                                c_sb[:, kb, e * N:(e + 1) * N], b8full[:, kb, e, :], 1.0,
                            )
                q_sb = pp.tile([64, B, N], bf16, name="q_sb")
                k_sb = pp.tile([64, B, N], bf16, name="k_sb")
                v_sb = pp.tile([128, B, 8, HPC, D + 1], bf16, name="v_sb")
                nc.vector.memset(v_sb[:, :, :, :, D], 1.0)

                for b in range(B):
                    x_t = wp.tile([128, 4, N], bf16, tag="xt", bufs=2)
                    nc.sync.dma_start(x_t[:, :, :], x_d[b].rearrange("(kc p) n -> p kc n", p=128))

                    qk_ps = psp.tile([128, N], fp32, tag="st", bufs=stb,
                                     padded_shape=stpad)
                    for kc in range(4):
                        for half in range(2):
                            sl = slice(512 * half, 512 * (half + 1))
                            nc.tensor.matmul(
                                qk_ps[:, sl],
                                wqk_sb[:, kc, :],
                                x_t[:, kc, sl],
                                start=(kc == 0),
                                stop=(kc == 3),
                            )
                    nc.vector.tensor_copy(q_sb[:, b, :], qk_ps[0:64, :])
                    nc.vector.tensor_copy(k_sb[:, b, :], qk_ps[64:128, :])

                    for tbq in range(2):
                        v_ps = psp.tile([128, 4, 2 * D], fp32, tag="st", bufs=stb,
                                        padded_shape=[128, 4, (N if split_st else 2 * N) // 4])
                        for tb4 in range(4):
                            tb = 4 * tbq + tb4
                            for kc in range(4):
                                nc.tensor.matmul(
                                    v_ps[:, tb4, :],
                                    x_t[:, kc, 128 * tb:128 * (tb + 1)],
                                    wv_sb[:, kc, :],
                                    start=(kc == 0),
                                    stop=(kc == 3),
                                )
                        nc.vector.tensor_copy(
                            v_sb[:, b, 4 * tbq:4 * (tbq + 1), :, 0:D],
                            v_ps[:, :, :].rearrange("p t (h d) -> p t h d", h=HPC),
                        )

                attnout_sb = pp.tile([128, B, N], bf16, name="attnout_sb")

                for b in range(B):
                    avh = [pav.tile([D + 1, N], fp32, tag=f"av{h}", bufs=1,
                                    padded_shape=[128, N], name=f"av{h}")
                           for h in range(HPC)]
                    for kb in range(8):
                        if split_st:
                            for h in range(HPC):
                                st = psp.tile([128, N], fp32, tag="st", bufs=stb,
                                              padded_shape=stpad)
                                for half in range(2):
                                    sl = slice(512 * half, 512 * (half + 1))
                                    nc.tensor.matmul(
                                        st[:, sl],
                                        k_sb[32 * h:32 * (h + 1), b, 128 * kb:128 * (kb + 1)],
                                        q_sb[32 * h:32 * (h + 1), b, sl],
                                        start=True,
                                        stop=True,
                                    )
                                at = wp.tile([128, N], bf16, tag="attn", bufs=at_bufs)
                                nc.scalar.activation(at[:, :], st[:, :], mybir.ActivationFunctionType.Exp)
                                if not skip_mult:
                                    nc.vector.tensor_mul(at[:, :], at[:, :],
                                                         c_sb[:, kb, h * N:(h + 1) * N])
                                for half in range(2):
                                    sl = slice(512 * half, 512 * (half + 1))
                                    nc.tensor.matmul(
                                        avh[h][:, sl],
                                        v_sb[:, b, kb, h, :],
                                        at[:, sl],
                                        start=(kb == 0),
                                        stop=(kb == 7),
                                    )
                            continue
                        st2 = psp.tile([128, 2 * N], fp32, tag="st", bufs=stb)
                        for h in range(HPC):
                            for half in range(2):
                                sl = slice(512 * half, 512 * (half + 1))
                                nc.tensor.matmul(
                                    st2[:, h * N + 512 * half:h * N + 512 * (half + 1)],
                                    k_sb[32 * h:32 * (h + 1), b, 128 * kb:128 * (kb + 1)],
                                    q_sb[32 * h:32 * (h + 1), b, sl],
                                    start=True,
                                    stop=True,
                                )
                        at = wp.tile([128, 2 * N], bf16, tag="attn", bufs=at_bufs)
                        nc.scalar.activation(at[:, :], st2[:, :], mybir.ActivationFunctionType.Exp)
                        if not skip_mult:
                            nc.vector.tensor_mul(at[:, :], at[:, :], c_sb[:, kb, :])
                        for h in range(HPC):
                            for half in range(2):
                                sl = slice(512 * half, 512 * (half + 1))
                                nc.tensor.matmul(
                                    avh[h][:, sl],
                                    v_sb[:, b, kb, h, :],
                                    at[:, h * N + 512 * half:h * N + 512 * (half + 1)],
                                    start=(kb == 0),
                                    stop=(kb == 7),
                                )
                    for h in range(HPC):
                        nc.vector.tensor_copy(
                            attnout_sb[64 * h:64 * h + D + 1, b, :],
                            avh[h][:, :],
                        )
                    nc.sync.dma_start(ao_in[b][0:D + 1], attnout_sb[0:D + 1, b, :])
                    nc.sync.dma_start(ao_in[b][D + 1:], attnout_sb[64:64 + D + 1, b, :])

                if not skip_bias and _rep + 1 < repeat:
                    emit_frontend()

                if collectives:
                    nc.gpsimd.collective_compute(
                        "AllToAll",
                        mybir.AluOpType.bypass,
                        replica_groups=RG,
                        ins=[ao_in.opt()],
                        outs=[ao_out.opt()],
                    )
                else:
                    nc.gpsimd.dma_start(ao_out.opt(), ao_in.opt())
                if not skip_bias and _rep + 1 < repeat:
                    emit_loads()

                den_sb = wp.tile([16, N], bf16, tag="den", bufs=1)
                nc.sync.dma_start(
                    den_sb[:, :],
                    ao_out.rearrange("s (h x) n -> (s h) x n", x=D + 1)[:, D, :],
                )
                rec_sb = wp.tile([16, N], bf16, tag="rec", bufs=1)
                with nc.allow_low_precision(reason="bf16 softmax denominators are within tolerance"):
                    nc.vector.reciprocal(rec_sb[:, :], den_sb[:, :])

                ao_sb = pp.tile([128, 4, N], bf16, name="ao_sb")
                for kc in range(4):
                    for j in range(2):
                        for h in range(HPC):
                            nc.sync.dma_start(
                                ao_sb[64 * j + 32 * h:64 * j + 32 * h + D, kc, :],
                                ao_out[2 * kc + j, (D + 1) * h:(D + 1) * h + D, :],
                            )
                for kc in range(4):
                    bc_ps = pav.tile([128, N], fp32, tag="av0", bufs=1)
                    for half in range(2):
                        sl = slice(512 * half, 512 * (half + 1))
                        nc.tensor.matmul(bc_ps[:, sl], e16_sb[:, kc, :], rec_sb[:, sl],
                                         start=True, stop=True)
                    nc.vector.tensor_mul(ao_sb[:, kc, :], ao_sb[:, kc, :], bc_ps[:, :])

                for mb in range(4):
                    o_ps = psp.tile([128, N], fp32, tag="st", bufs=stb,
                                    padded_shape=stpad)
                    for kc in range(4):
                        for half in range(2):
                            sl = slice(512 * half, 512 * (half + 1))
                            nc.tensor.matmul(
                                o_ps[:, sl],
                                wo_sb[:, kc, 128 * mb:128 * (mb + 1)],
                                ao_sb[:, kc, sl],
                                start=(kc == 0),
                                stop=(kc == 3),
                            )
                    o_sb = wp.tile([128, N], fp32, tag="osb", bufs=2)
                    nc.vector.tensor_scalar_add(o_sb[:, :], o_ps[:, :], bo_sb[:, mb:mb + 1])
                    nc.sync.dma_start(out_d[128 * mb:128 * (mb + 1), :], o_sb[:, :])

    nc.compile()
    return nc


# ============================================================================
# host-side prep
# ============================================================================

def _common_prep(inputs):
    x = np.asarray(inputs["x"], np.float32).reshape(B, C, N)
    w_qkv = np.asarray(inputs["w_qkv"], np.float32)
    w_out = np.asarray(inputs["w_out"], np.float32)
    b_out = np.asarray(inputs["b_out"], np.float32)

    x_bf = np.ascontiguousarray(x.astype(BF))
    wq = w_qkv[0:OUP]
    wk = w_qkv[OUP:2 * OUP]
    wv = w_qkv[2 * OUP:3 * OUP]
    w_outT = np.ascontiguousarray(w_out.T.astype(BF))
    b_out4 = np.ascontiguousarray(b_out.reshape(4, 128).T.astype(np.float32))
    e16_bf = np.ascontiguousarray(E16.astype(BF))

    per_core = []
    for c in range(NCORES):
        h0, h1 = 2 * c, 2 * c + 1
        wqk_cols = np.concatenate(
            [
                wq[h0 * D:(h0 + 1) * D] * SCALE,
                wq[h1 * D:(h1 + 1) * D] * SCALE,
                wk[h0 * D:(h0 + 1) * D],
                wk[h1 * D:(h1 + 1) * D],
            ],
            axis=0,
        )
        w_qkT = np.ascontiguousarray(wqk_cols.T.astype(BF))
        wv_cols = np.concatenate(
            [wv[h0 * D:(h0 + 1) * D], wv[h1 * D:(h1 + 1) * D]], axis=0
        )
        w_vT = np.ascontiguousarray(wv_cols.T.astype(BF))
        per_core.append({
            "x": x_bf,
            "w_qkT": w_qkT,
            "w_vT": w_vT,
            "w_outT": w_outT,
            "b_out4": b_out4,
            "e16": e16_bf,
        })
    return per_core


def _prepare_in_maps(inputs):
    """v4 fast path: per-core exp'd, window-expanded bias tables."""
    table = np.asarray(inputs["bias_table"], np.float32)
    c_table = np.exp(table.astype(np.float64)).astype(np.float32)  # [3969, 16]
    # window expansion: idx[kx, u, qx] = 63*u + qx + 31 - kx  (all in range)
    kx = np.arange(32)[:, None, None]
    u = np.arange(63)[None, :, None]
    qx = np.arange(32)[None, None, :]
    widx = (63 * u + qx + 31 - kx).reshape(-1)  # [32*63*32]
    in_maps = _common_prep(inputs)
    for c in range(NCORES):
        be = np.stack([c_table[widx, 2 * c], c_table[widx, 2 * c + 1]])
        be = np.tile(be[:, None, :], (1, 4, 1)).reshape(2, 258048)
        in_maps[c]["bias_e"] = np.ascontiguousarray(be.astype(BF))
    return in_maps


def _prepare_in_maps_gather(inputs):
    """v3 fallback: raw table + i16 gather indices."""
    table = np.asarray(inputs["bias_table"], np.float32)
    ridx = np.asarray(inputs["rel_index"]).astype(np.int64).reshape(N, N)
    tab_rep = np.ascontiguousarray(table.T[np.arange(128) % HEADS].astype(np.float32))
    in_maps = _common_prep(inputs)
    for c in range(NCORES):
        sl = ridx[:, KCH * c:KCH * (c + 1)]
        idxw = np.empty((128, N), np.int16)
        for g in range(8):
            arr = sl[:, 16 * g:16 * (g + 1)].T.reshape(-1)
            idxw[16 * g:16 * (g + 1)] = arr.reshape(N, 16).T
        in_maps[c]["table"] = tab_rep
        in_maps[c]["idx"] = np.ascontiguousarray(idxw)
    return in_maps


def _get_graph(**kw):
    key = ("v4",) + tuple(sorted(kw.items()))
    if key not in _GRAPH_CACHE:
        _GRAPH_CACHE[key] = _build_graph(**kw)
    return _GRAPH_CACHE[key]


def _get_graph_gather(**kw):
    key = ("v3",) + tuple(sorted(kw.items()))
    if key not in _GRAPH_CACHE:
        _GRAPH_CACHE[key] = _build_graph_gather(**kw)
    return _GRAPH_CACHE[key]


def run_on_hw(inputs, trace=False, **kw):
    from concourse.bass_utils import run_bass_kernel_spmd

    nc = _get_graph()
    in_maps = _prepare_in_maps(inputs)
    return run_bass_kernel_spmd(nc, in_maps, core_ids=list(range(NCORES)), trace=trace, **kw)


def run_on_hw_gather(inputs, trace=False, **kw):
    from concourse.bass_utils import run_bass_kernel_spmd

    nc = _get_graph_gather()
    in_maps = _prepare_in_maps_gather(inputs)
    return run_bass_kernel_spmd(nc, in_maps, core_ids=list(range(NCORES)), trace=trace, **kw)


def kernel(**inputs) -> np.ndarray:
    ridx = np.asarray(inputs["rel_index"]).ravel()
    if ridx.shape == (N * N,) and np.array_equal(ridx, _toeplitz_rel_index()):
        res = run_on_hw(inputs).results
    else:
        res = run_on_hw_gather(inputs).results
    out = np.stack([np.asarray(res[c]["out"], np.float32) for c in range(NCORES)])
    return out.reshape(B, OUP, 32, 32)


if __name__ == "__main__":
    _get_graph()
    print("graph built + compiled OK")


# revision 17
# speedup vs baseline: 1.2526x; 1.2526x over previous
"""Trainium2 Bass kernel for nn_Attention_14955076125505.

Windowed self-attention with relative-position bias:
  x:(8,512,32,32) -> qkv -> 16-head attention(N=1024, d=32) + bias_table[rel_index]
  -> out proj -> (8,512,32,32)

Sharding (8 NeuronCores), v4:
  - tensor-parallel over heads: core c owns heads (2c, 2c+1) for qkv + attention.
  - bias: rel_index from the reference is the deterministic 2D relative-
    position (Toeplitz) layout: idx(q,k) = 63*(qy-ky+31) + (qx-kx+31).
    kernel() verifies this host-side; when it holds, each per-(kb) bias tile
    c[key=128kb+p, head e, q] = exp(table[idx, h]) is a strided window read
    over the 63x63 exp'd table, fetched by ONE DMA per kb with a
    hand-built access pattern (negative strides on the key dims).  This
    replaces the previous GpSimd ap_gather + fp8 + AllToAll frontend
    (~300us/rep).  If rel_index ever fails the check, a gather-based
    fallback graph (v3) is used instead.
  - attention output is shipped UNNORMALIZED (+ the per-(head,q) softmax
    denominator as a 33rd row) through an AllToAll that re-shards by
    batch; the batch owner computes reciprocals, broadcasts them over d via a
    small matmul, normalizes, and runs the output projection for its batch.
Compute dtype bf16 on the TensorEngine (f32 PSUM accumulation), exp on ScalarE,
bias multiply on VectorE.
"""

import sys

if "/opt/trn_rl_repo" not in sys.path:
    sys.path.insert(0, "/opt/trn_rl_repo")

import numpy as np
import ml_dtypes

B = 8
C = 512
N = 1024  # H*W
HEADS = 16
D = 32
OUP = 512
TABLE = 3969
NCORES = 8
HPC = 2  # heads per core
KCH = 128  # keys gathered per core (fallback path)
SCALE = D ** -0.5

BF = ml_dtypes.bfloat16

_GRAPH_CACHE = {}

# owner-side reciprocal broadcast selectors, one per kc pair of source cores:
# row r of ao_sb[:, kc, :] is (core 2kc + r//64, head-in-core (r%64)//32, d),
# i.e. global head 2*(2kc + r//64) + (r%64)//32; E16[kc][head, r] = 1 selects
# that head's reciprocal row.
E16 = np.zeros((16, 4, 128), np.float32)
for _kc in range(4):
    for _r in range(128):
        _head = 2 * (2 * _kc + _r // 64) + (_r % 64) // 32
        E16[_head, _kc, _r] = 1.0


def _toeplitz_rel_index():
    yy, xx = np.meshgrid(np.arange(32), np.arange(32), indexing="ij")
    coords = np.stack([yy.ravel(), xx.ravel()])
    rel = coords[:, :, None] - coords[:, None, :]
    rel[0] += 31
    rel[1] += 31
    rel[0] *= 63
    return rel.sum(0).ravel()  # (N*N,) q-major


# ============================================================================
# v4 graph: DMA-window bias (Toeplitz fast path)
# ============================================================================

def _build_graph(repeat=1, collectives=True, num_devices=NCORES, skip_bias=False,
                 skip_mult=False, at_bufs=4, st_bufs=1, split_st=False,
                 mult_pool=0):
    import concourse.bass as bass
    import concourse.mybir as mybir
    import concourse.tile as tile
    from concourse import bacc

    fp32 = mybir.dt.float32
    bf16 = mybir.dt.bfloat16
    stb = 2 if split_st else st_bufs
    stpad = [128, N if split_st else 2 * N]

    nc = bacc.Bacc(
        "TRN2",
        target_bir_lowering=False,
        debug=False,
        enable_asserts=True,
        num_devices=num_devices,
    )

    # ---- kernel I/O (per-core shards, prepared host-side) ----
    x_d = nc.dram_tensor("x", [B, C, N], bf16, kind="ExternalInput").ap()
    wqk_d = nc.dram_tensor("w_qkT", [C, 128], bf16, kind="ExternalInput").ap()
    wv_d = nc.dram_tensor("w_vT", [C, 2 * D], bf16, kind="ExternalInput").ap()
    wo_d = nc.dram_tensor("w_outT", [OUP, OUP], bf16, kind="ExternalInput").ap()
    bo_d = nc.dram_tensor("b_out4", [128, 4], fp32, kind="ExternalInput").ap()
    e16_d = nc.dram_tensor("e16", [16, 4, 128], bf16, kind="ExternalInput").ap()
    # exp'd bias table for this core's 2 heads, window-expanded and
    # replicated 4x over ky (see _prepare_in_maps):
    # bias_e[e, 64512*ky + 2016*kx + 32*u + qx] = exp(table[63*u + qx +
    # 31 - kx, 2c+e]); per (kb, e) the c tile is then a 3-dim all-positive
    # strided DMA with 1024-element contiguous runs.
    be_d = nc.dram_tensor("bias_e", [2, 258048], bf16, kind="ExternalInput").ap()
    out_d = nc.dram_tensor("out", [OUP, N], fp32, kind="ExternalOutput").ap()

    # attention-out a2a: 66 rows = 2 heads x (32 out + 1 denominator)
    ao_in = nc.dram_tensor("ao_a2a_in", [NCORES, 2 * (D + 1), N], bf16).ap()
    ao_out = nc.dram_tensor("ao_a2a_out", [NCORES, 2 * (D + 1), N], bf16).ap()

    RG = [list(range(NCORES))]

    def bias_window_ap(kb, e):
        """Source AP over bias_e for the c tile of key-block kb, head e:
        element (p=(ky',kx), qy, qx) reads the exp'd bias for
        key=(4*kb+ky', kx), query=(qy, qx): addr = e*258048 + 64512*ky' +
        2016*kx + 32*(qy + 31 - 4*kb - ky') + qx, i.e. ky' stride
        64512-32 = 64480 against the ky-replicated table."""
        return bass.AP(
            tensor=be_d.tensor,
            offset=e * 258048 + (31 - 4 * kb) * 32,
            ap=[[64480, 4], [2016, 32], [1, 1024]],
        )

    with tile.TileContext(nc) as tc:
        with (
            tc.tile_pool(name="const", bufs=1) as cp,
            tc.tile_pool(name="persist", bufs=1) as pp,
            tc.tile_pool(name="work", bufs=3) as wp,
            tc.tile_pool(name="psum", bufs=2, space="PSUM") as psp,
            tc.tile_pool(name="psav", bufs=2, space="PSUM") as pav,
        ):
            # ================= constants =================
            wqk_sb = cp.tile([128, 4, 128], bf16)
            nc.sync.dma_start(wqk_sb[:, :, :], wqk_d.rearrange("(kc p) m -> p kc m", p=128))
            wv_sb = cp.tile([128, 4, 2 * D], bf16)
            nc.sync.dma_start(wv_sb[:, :, :], wv_d.rearrange("(kc p) m -> p kc m", p=128))
            wo_sb = cp.tile([128, 4, OUP], bf16)
            nc.sync.dma_start(wo_sb[:, :, :], wo_d.rearrange("(kc p) m -> p kc m", p=128))
            bo_sb = cp.tile([128, 4], fp32)
            nc.sync.dma_start(bo_sb[:, :], bo_d)
            e16_sb = cp.tile([16, 4, 128], bf16)
            nc.sync.dma_start(e16_sb[:, :, :], e16_d)

            for _rep in range(repeat):
                # bias tiles for all 8 key-blocks (shared across batches)
                c_sb = pp.tile([128, 8, HPC * N], bf16, name="c_sb")
                if skip_bias:
                    nc.vector.memset(c_sb[:, :, :], 1.0)
                else:
                    for kb in range(8):
                        for e in range(HPC):
                            nc.sync.dma_start(c_sb[:, kb, e * N:(e + 1) * N],
                                              bias_window_ap(kb, e))

                # ================= qkv projection (all 8 batches) =================
                q_sb = pp.tile([64, B, N], bf16, name="q_sb")
                k_sb = pp.tile([64, B, N], bf16, name="k_sb")
                v_sb = pp.tile([128, B, 8, HPC, D + 1], bf16, name="v_sb")
                nc.vector.memset(v_sb[:, :, :, :, D], 1.0)

                for b in range(B):
                    x_t = wp.tile([128, 4, N], bf16, tag="xt", bufs=2)
                    nc.sync.dma_start(x_t[:, :, :], x_d[b].rearrange("(kc p) n -> p kc n", p=128))

                    qk_ps = psp.tile([128, N], fp32, tag="st", bufs=stb,
                                     padded_shape=stpad)
                    for kc in range(4):
                        for half in range(2):
                            sl = slice(512 * half, 512 * (half + 1))
                            nc.tensor.matmul(
                                qk_ps[:, sl],
                                wqk_sb[:, kc, :],
                                x_t[:, kc, sl],
                                start=(kc == 0),
                                stop=(kc == 3),
                            )
                    nc.vector.tensor_copy(q_sb[:, b, :], qk_ps[0:64, :])
                    nc.vector.tensor_copy(k_sb[:, b, :], qk_ps[64:128, :])

                    for tbq in range(2):
                        v_ps = psp.tile([128, 4, 2 * D], fp32, tag="st", bufs=stb,
                                        padded_shape=[128, 4, (N if split_st else 2 * N) // 4])
                        for tb4 in range(4):
                            tb = 4 * tbq + tb4
                            for kc in range(4):
                                nc.tensor.matmul(
                                    v_ps[:, tb4, :],
                                    x_t[:, kc, 128 * tb:128 * (tb + 1)],
                                    wv_sb[:, kc, :],
                                    start=(kc == 0),
                                    stop=(kc == 3),
                                )
                        nc.vector.tensor_copy(
                            v_sb[:, b, 4 * tbq:4 * (tbq + 1), :, 0:D],
                            v_ps[:, :, :].rearrange("p t (h d) -> p t h d", h=HPC),
                        )

                # ================= attention =================
                # attnout_sb rows 64h+d: unnormalized out (d<32) + denominator
                # (d=32); head blocks start at partitions 0/64 (32-aligned)
                attnout_sb = pp.tile([128, B, N], bf16, name="attnout_sb")

                for b in range(B):
                    avh = [pav.tile([D + 1, N], fp32, tag=f"av{h}", bufs=1,
                                    padded_shape=[128, N], name=f"av{h}")
                           for h in range(HPC)]
                    for kb in range(8):
                        if split_st:
                            # fine tiles: per-head scores, 2-buf PE<->Act ping-pong
                            for h in range(HPC):
                                st = psp.tile([128, N], fp32, tag="st", bufs=stb,
                                              padded_shape=stpad)
                                for half in range(2):
                                    sl = slice(512 * half, 512 * (half + 1))
                                    nc.tensor.matmul(
                                        st[:, sl],
                                        k_sb[32 * h:32 * (h + 1), b, 128 * kb:128 * (kb + 1)],
                                        q_sb[32 * h:32 * (h + 1), b, sl],
                                        start=True,
                                        stop=True,
                                    )
                                at = wp.tile([128, N], bf16, tag="attn", bufs=at_bufs)
                                nc.scalar.activation(at[:, :], st[:, :], mybir.ActivationFunctionType.Exp)
                                if not skip_mult:
                                    eng = nc.gpsimd if (mult_pool and h == 1) else nc.vector
                                    eng.tensor_mul(at[:, :], at[:, :],
                                                   c_sb[:, kb, h * N:(h + 1) * N])
                                for half in range(2):
                                    sl = slice(512 * half, 512 * (half + 1))
                                    nc.tensor.matmul(
                                        avh[h][:, sl],
                                        v_sb[:, b, kb, h, :],
                                        at[:, sl],
                                        start=(kb == 0),
                                        stop=(kb == 7),
                                    )
                            continue
                        st2 = psp.tile([128, 2 * N], fp32, tag="st", bufs=stb)
                        for h in range(HPC):
                            for half in range(2):
                                sl = slice(512 * half, 512 * (half + 1))
                                nc.tensor.matmul(
                                    st2[:, h * N + 512 * half:h * N + 512 * (half + 1)],
                                    k_sb[32 * h:32 * (h + 1), b, 128 * kb:128 * (kb + 1)],
                                    q_sb[32 * h:32 * (h + 1), b, sl],
                                    start=True,
                                    stop=True,
                                )
                        at = wp.tile([128, 2 * N], bf16, tag="attn", bufs=at_bufs)
                        nc.scalar.activation(at[:, :], st2[:, :], mybir.ActivationFunctionType.Exp)
                        if not skip_mult:
                            if mult_pool:
                                nc.vector.tensor_mul(at[:, 0:N], at[:, 0:N], c_sb[:, kb, 0:N])
                                nc.gpsimd.tensor_mul(at[:, N:], at[:, N:], c_sb[:, kb, N:])
                            else:
                                nc.vector.tensor_mul(at[:, :], at[:, :], c_sb[:, kb, :])
                        for h in range(HPC):
                            for half in range(2):
                                sl = slice(512 * half, 512 * (half + 1))
                                nc.tensor.matmul(
                                    avh[h][:, sl],
                                    v_sb[:, b, kb, h, :],
                                    at[:, h * N + 512 * half:h * N + 512 * (half + 1)],
                                    start=(kb == 0),
                                    stop=(kb == 7),
                                )
                    for h in range(HPC):
                        nc.vector.tensor_copy(
                            attnout_sb[64 * h:64 * h + D + 1, b, :],
                            avh[h][:, :],
                        )
                    nc.sync.dma_start(ao_in[b][0:D + 1], attnout_sb[0:D + 1, b, :])
                    nc.sync.dma_start(ao_in[b][D + 1:], attnout_sb[64:64 + D + 1, b, :])

                # ================= all-to-all: heads -> batch =================
                if collectives:
                    nc.gpsimd.collective_compute(
                        "AllToAll",
                        mybir.AluOpType.bypass,
                        replica_groups=RG,
                        ins=[ao_in.opt()],
                        outs=[ao_out.opt()],
                    )
                else:
                    nc.gpsimd.dma_start(ao_out.opt(), ao_in.opt())

                # ================= normalize + output projection (my batch) =======
                den_sb = wp.tile([16, N], bf16, tag="den", bufs=1)
                nc.sync.dma_start(
                    den_sb[:, :],
                    ao_out.rearrange("s (h x) n -> (s h) x n", x=D + 1)[:, D, :],
                )
                rec_sb = wp.tile([16, N], bf16, tag="rec", bufs=1)
                with nc.allow_low_precision(reason="bf16 softmax denominators are within tolerance"):
                    nc.vector.reciprocal(rec_sb[:, :], den_sb[:, :])

                ao_sb = pp.tile([128, 4, N], bf16, name="ao_sb")
                for kc in range(4):
                    for j in range(2):
                        for h in range(HPC):
                            nc.sync.dma_start(
                                ao_sb[64 * j + 32 * h:64 * j + 32 * h + D, kc, :],
                                ao_out[2 * kc + j, (D + 1) * h:(D + 1) * h + D, :],
                            )
                for kc in range(4):
                    bc_ps = pav.tile([128, N], fp32, tag="av0", bufs=1)
                    for half in range(2):
                        sl = slice(512 * half, 512 * (half + 1))
                        nc.tensor.matmul(bc_ps[:, sl], e16_sb[:, kc, :], rec_sb[:, sl],
                                         start=True, stop=True)
                    nc.vector.tensor_mul(ao_sb[:, kc, :], ao_sb[:, kc, :], bc_ps[:, :])

                for mb in range(4):
                    o_ps = psp.tile([128, N], fp32, tag="st", bufs=stb,
                                    padded_shape=stpad)
                    for kc in range(4):
                        for half in range(2):
                            sl = slice(512 * half, 512 * (half + 1))
                            nc.tensor.matmul(
                                o_ps[:, sl],
                                wo_sb[:, kc, 128 * mb:128 * (mb + 1)],
                                ao_sb[:, kc, sl],
                                start=(kc == 0),
                                stop=(kc == 3),
                            )
                    o_sb = wp.tile([128, N], fp32, tag="osb", bufs=2)
                    nc.vector.tensor_scalar_add(o_sb[:, :], o_ps[:, :], bo_sb[:, mb:mb + 1])
                    nc.sync.dma_start(out_d[128 * mb:128 * (mb + 1), :], o_sb[:, :])

    nc.compile()
    return nc


# ============================================================================
# v3 fallback graph: GpSimd gather + fp8 + AllToAll bias frontend
# ============================================================================

def _build_graph_gather(repeat=1, collectives=True, num_devices=NCORES, skip_bias=False,
                        skip_mult=False, fp8_bias=True, at_bufs=4, st_bufs=1,
                        split_st=False, skip_gather=False, nchunk=4):
    import concourse.bass as bass
    import concourse.mybir as mybir
    import concourse.tile as tile
    from concourse import bacc

    fp32 = mybir.dt.float32
    bf16 = mybir.dt.bfloat16
    f8 = mybir.dt.float8e4
    i16 = mybir.dt.int16
    bias_dt = f8 if fp8_bias else bf16
    stb = 2 if split_st else st_bufs
    stpad = [128, N if split_st else 2 * N]

    nc = bacc.Bacc(
        "TRN2",
        target_bir_lowering=False,
        debug=False,
        enable_asserts=True,
        num_devices=num_devices,
    )

    x_d = nc.dram_tensor("x", [B, C, N], bf16, kind="ExternalInput").ap()
    wqk_d = nc.dram_tensor("w_qkT", [C, 128], bf16, kind="ExternalInput").ap()
    wv_d = nc.dram_tensor("w_vT", [C, 2 * D], bf16, kind="ExternalInput").ap()
    wo_d = nc.dram_tensor("w_outT", [OUP, OUP], bf16, kind="ExternalInput").ap()
    bo_d = nc.dram_tensor("b_out4", [128, 4], fp32, kind="ExternalInput").ap()
    tab_d = nc.dram_tensor("table", [128, TABLE], fp32, kind="ExternalInput").ap()
    idx_d = nc.dram_tensor("idx", [128, N], i16, kind="ExternalInput").ap()
    e16_d = nc.dram_tensor("e16", [16, 4, 128], bf16, kind="ExternalInput").ap()
    out_d = nc.dram_tensor("out", [OUP, N], fp32, kind="ExternalOutput").ap()

    ebc_in = nc.dram_tensor("ebc_a2a_in", [16, 8, 16384], bias_dt).ap()
    ebc_out = nc.dram_tensor("ebc_a2a_out", [NCORES, HPC, 8, 16384], bias_dt).ap()
    ao_in = nc.dram_tensor("ao_a2a_in", [NCORES, 2 * (D + 1), N], bf16).ap()
    ao_out = nc.dram_tensor("ao_a2a_out", [NCORES, 2 * (D + 1), N], bf16).ap()

    RG = [list(range(NCORES))]

    with tile.TileContext(nc) as tc:
        with (
            tc.tile_pool(name="const", bufs=1) as cp,
            tc.tile_pool(name="persist", bufs=1) as pp,
            tc.tile_pool(name="work", bufs=3) as wp,
            tc.tile_pool(name="gather", bufs=1) as gp,
            tc.tile_pool(name="psum", bufs=2, space="PSUM") as psp,
            tc.tile_pool(name="psav", bufs=2, space="PSUM") as pav,
        ):
            tab_sb = cp.tile([128, TABLE], fp32)
            nc.sync.dma_start(tab_sb[:, :], tab_d)
            idx_sb = cp.tile([128, N], i16)
            nc.sync.dma_start(idx_sb[:, :], idx_d)
            wqk_sb = cp.tile([128, 4, 128], bf16)
            nc.sync.dma_start(wqk_sb[:, :, :], wqk_d.rearrange("(kc p) m -> p kc m", p=128))
            wv_sb = cp.tile([128, 4, 2 * D], bf16)
            nc.sync.dma_start(wv_sb[:, :, :], wv_d.rearrange("(kc p) m -> p kc m", p=128))
            wo_sb = cp.tile([128, 4, OUP], bf16)
            nc.sync.dma_start(wo_sb[:, :, :], wo_d.rearrange("(kc p) m -> p kc m", p=128))
            bo_sb = cp.tile([128, 4], fp32)
            nc.sync.dma_start(bo_sb[:, :], bo_d)
            e16_sb = cp.tile([16, 4, 128], bf16)
            nc.sync.dma_start(e16_sb[:, :, :], e16_d)

            b8full = cp.tile([128, 8, HPC, N], bias_dt)

            NIDX = 16 * N
            NCHUNK = nchunk
            CH = NIDX // NCHUNK

            def emit_frontend():
                for ch in range(NCHUNK):
                    gath = gp.tile([128, CH], fp32, tag="gath", bufs=1)
                    if skip_gather:
                        nc.gpsimd.memset(gath[:, :], 0.01)
                    else:
                        nc.gpsimd.ap_gather(
                            out_ap=gath[:, :],
                            in_ap=tab_sb[:, :],
                            idxs_ap=idx_sb[:, ch * (CH // 16):(ch + 1) * (CH // 16)],
                            channels=128,
                            num_elems=TABLE,
                            d=1,
                            num_idxs=CH,
                        )
                    b8 = gp.tile([128, CH], bias_dt, tag="b8", bufs=1)
                    nc.gpsimd.tensor_copy(b8[:, :], gath[:, :])
                    nc.gpsimd.dma_start(
                        ebc_in.rearrange("r g (ch f) -> g r ch f", f=CH)[:, :, ch, :],
                        b8[:, :],
                    )
                if collectives:
                    nc.gpsimd.collective_compute(
                        "AllToAll",
                        mybir.AluOpType.bypass,
                        replica_groups=RG,
                        ins=[ebc_in.opt()],
                        outs=[ebc_out.opt()],
                    )
                else:
                    nc.gpsimd.dma_start(
                        ebc_out.rearrange("s e g f -> (s e) g f"), ebc_in.opt())

            def emit_loads():
                for kb in range(8):
                    for e in range(HPC):
                        nc.gpsimd.dma_start(
                            b8full[:, kb, e, :],
                            ebc_out[kb, e].rearrange("g (fq q) -> (g fq) q", q=N),
                        )

            for _rep in range(repeat):
                c_sb = pp.tile([128, 8, HPC * N], bf16, name="c_sb")
                if skip_bias:
                    nc.vector.memset(c_sb[:, :, :], 1.0)
                else:
                    if _rep == 0:
                        emit_frontend()
                        emit_loads()
                    for kb in range(8):
                        for e in range(HPC):
                            nc.vector.tensor_scalar_add(
                                c_sb[:, kb, e * N:(e + 1) * N], b8full[:, kb, e, :], 1.0,
                            )
                q_sb = pp.tile([64, B, N], bf16, name="q_sb")
                k_sb = pp.tile([64, B, N], bf16, name="k_sb")
                v_sb = pp.tile([128, B, 8, HPC, D + 1], bf16, name="v_sb")
                nc.vector.memset(v_sb[:, :, :, :, D], 1.0)

                for b in range(B):
                    x_t = wp.tile([128, 4, N], bf16, tag="xt", bufs=2)
                    nc.sync.dma_start(x_t[:, :, :], x_d[b].rearrange("(kc p) n -> p kc n", p=128))

                    qk_ps = psp.tile([128, N], fp32, tag="st", bufs=stb,
                                     padded_shape=stpad)
                    for kc in range(4):
                        for half in range(2):
                            sl = slice(512 * half, 512 * (half + 1))
                            nc.tensor.matmul(
                                qk_ps[:, sl],
                                wqk_sb[:, kc, :],
                                x_t[:, kc, sl],
                                start=(kc == 0),
                                stop=(kc == 3),
                            )
                    nc.vector.tensor_copy(q_sb[:, b, :], qk_ps[0:64, :])
                    nc.vector.tensor_copy(k_sb[:, b, :], qk_ps[64:128, :])

                    for tbq in range(2):
                        v_ps = psp.tile([128, 4, 2 * D], fp32, tag="st", bufs=stb,
                                        padded_shape=[128, 4, (N if split_st else 2 * N) // 4])
                        for tb4 in range(4):
                            tb = 4 * tbq + tb4
                            for kc in range(4):
                                nc.tensor.matmul(
                                    v_ps[:, tb4, :],
                                    x_t[:, kc, 128 * tb:128 * (tb + 1)],
                                    wv_sb[:, kc, :],
                                    start=(kc == 0),
                                    stop=(kc == 3),
                                )
                        nc.vector.tensor_copy(
                            v_sb[:, b, 4 * tbq:4 * (tbq + 1), :, 0:D],
                            v_ps[:, :, :].rearrange("p t (h d) -> p t h d", h=HPC),
                        )

                attnout_sb = pp.tile([128, B, N], bf16, name="attnout_sb")

                for b in range(B):
                    avh = [pav.tile([D + 1, N], fp32, tag=f"av{h}", bufs=1,
                                    padded_shape=[128, N], name=f"av{h}")
                           for h in range(HPC)]
                    for kb in range(8):
                        if split_st:
                            for h in range(HPC):
                                st = psp.tile([128, N], fp32, tag="st", bufs=stb,
                                              padded_shape=stpad)
                                for half in range(2):
                                    sl = slice(512 * half, 512 * (half + 1))
                                    nc.tensor.matmul(
                                        st[:, sl],
                                        k_sb[32 * h:32 * (h + 1), b, 128 * kb:128 * (kb + 1)],
                                        q_sb[32 * h:32 * (h + 1), b, sl],
                                        start=True,
                                        stop=True,
                                    )
                                at = wp.tile([128, N], bf16, tag="attn", bufs=at_bufs)
                                nc.scalar.activation(at[:, :], st[:, :], mybir.ActivationFunctionType.Exp)
                                if not skip_mult:
                                    nc.vector.tensor_mul(at[:, :], at[:, :],
                                                         c_sb[:, kb, h * N:(h + 1) * N])
                                for half in range(2):
                                    sl = slice(512 * half, 512 * (half + 1))
                                    nc.tensor.matmul(
                                        avh[h][:, sl],
                                        v_sb[:, b, kb, h, :],
                                        at[:, sl],
                                        start=(kb == 0),
                                        stop=(kb == 7),
                                    )
                            continue
                        st2 = psp.tile([128, 2 * N], fp32, tag="st", bufs=stb)
                        for h in range(HPC):
                            for half in range(2):
                                sl = slice(512 * half, 512 * (half + 1))
                                nc.tensor.matmul(
                                    st2[:, h * N + 512 * half:h * N + 512 * (half + 1)],
                                    k_sb[32 * h:32 * (h + 1), b, 128 * kb:128 * (kb + 1)],
                                    q_sb[32 * h:32 * (h + 1), b, sl],
                                    start=True,
                                    stop=True,
                                )
                        at = wp.tile([128, 2 * N], bf16, tag="attn", bufs=at_bufs)
                        nc.scalar.activation(at[:, :], st2[:, :], mybir.ActivationFunctionType.Exp)
                        if not skip_mult:
                            nc.vector.tensor_mul(at[:, :], at[:, :], c_sb[:, kb, :])
                        for h in range(HPC):
                            for half in range(2):
                                sl = slice(512 * half, 512 * (half + 1))
                                nc.tensor.matmul(
                                    avh[h][:, sl],
                                    v_sb[:, b, kb, h, :],
                                    at[:, h * N + 512 * half:h * N + 512 * (half + 1)],
                                    start=(kb == 0),
                                    stop=(kb == 7),
                                )
                    for h in range(HPC):
                        nc.vector.tensor_copy(
                            attnout_sb[64 * h:64 * h + D + 1, b, :],
                            avh[h][:, :],
                        )
                    nc.sync.dma_start(ao_in[b][0:D + 1], attnout_sb[0:D + 1, b, :])
                    nc.sync.dma_start(ao_in[b][D + 1:], attnout_sb[64:64 + D + 1, b, :])

                if not skip_bias and _rep + 1 < repeat:
                    emit_frontend()

                if collectives:
                    nc.gpsimd.collective_compute(
                        "AllToAll",
                        mybir.AluOpType.bypass,
                        replica_groups=RG,
                        ins=[ao_in.opt()],
                        outs=[ao_out.opt()],
                    )
                else:
                    nc.gpsimd.dma_start(ao_out.opt(), ao_in.opt())
                if not skip_bias and _rep + 1 < repeat:
                    emit_loads()

                den_sb = wp.tile([16, N], bf16, tag="den", bufs=1)
                nc.sync.dma_start(
                    den_sb[:, :],
                    ao_out.rearrange("s (h x) n -> (s h) x n", x=D + 1)[:, D, :],
                )
                rec_sb = wp.tile([16, N], bf16, tag="rec", bufs=1)
                with nc.allow_low_precision(reason="bf16 softmax denominators are within tolerance"):
                    nc.vector.reciprocal(rec_sb[:, :], den_sb[:, :])

                ao_sb = pp.tile([128, 4, N], bf16, name="ao_sb")
                for kc in range(4):
                    for j in range(2):
                        for h in range(HPC):
                            nc.sync.dma_start(
                                ao_sb[64 * j + 32 * h:64 * j + 32 * h + D, kc, :],
                                ao_out[2 * kc + j, (D + 1) * h:(D + 1) * h + D, :],
                            )
                for kc in range(4):
                    bc_ps = pav.tile([128, N], fp32, tag="av0", bufs=1)
                    for half in range(2):
                        sl = slice(512 * half, 512 * (half + 1))
                        nc.tensor.matmul(bc_ps[:, sl], e16_sb[:, kc, :], rec_sb[:, sl],
                                         start=True, stop=True)
                    nc.vector.tensor_mul(ao_sb[:, kc, :], ao_sb[:, kc, :], bc_ps[:, :])

                for mb in range(4):
                    o_ps = psp.tile([128, N], fp32, tag="st", bufs=stb,
                                    padded_shape=stpad)
                    for kc in range(4):
                        for half in range(2):
                            sl = slice(512 * half, 512 * (half + 1))
                            nc.tensor.matmul(
                                o_ps[:, sl],
                                wo_sb[:, kc, 128 * mb:128 * (mb + 1)],
                                ao_sb[:, kc, sl],
                                start=(kc == 0),
                                stop=(kc == 3),
                            )
                    o_sb = wp.tile([128, N], fp32, tag="osb", bufs=2)
                    nc.vector.tensor_scalar_add(o_sb[:, :], o_ps[:, :], bo_sb[:, mb:mb + 1])
                    nc.sync.dma_start(out_d[128 * mb:128 * (mb + 1), :], o_sb[:, :])

    nc.compile()
    return nc


# ============================================================================
# host-side prep
# ============================================================================

def _common_prep(inputs):
    x = np.asarray(inputs["x"], np.float32).reshape(B, C, N)
    w_qkv = np.asarray(inputs["w_qkv"], np.float32)
    w_out = np.asarray(inputs["w_out"], np.float32)
    b_out = np.asarray(inputs["b_out"], np.float32)

    x_bf = np.ascontiguousarray(x.astype(BF))
    wq = w_qkv[0:OUP]
    wk = w_qkv[OUP:2 * OUP]
    wv = w_qkv[2 * OUP:3 * OUP]
    w_outT = np.ascontiguousarray(w_out.T.astype(BF))
    b_out4 = np.ascontiguousarray(b_out.reshape(4, 128).T.astype(np.float32))
    e16_bf = np.ascontiguousarray(E16.astype(BF))

    per_core = []
    for c in range(NCORES):
        h0, h1 = 2 * c, 2 * c + 1
        wqk_cols = np.concatenate(
            [
                wq[h0 * D:(h0 + 1) * D] * SCALE,
                wq[h1 * D:(h1 + 1) * D] * SCALE,
                wk[h0 * D:(h0 + 1) * D],
                wk[h1 * D:(h1 + 1) * D],
            ],
            axis=0,
        )
        w_qkT = np.ascontiguousarray(wqk_cols.T.astype(BF))
        wv_cols = np.concatenate(
            [wv[h0 * D:(h0 + 1) * D], wv[h1 * D:(h1 + 1) * D]], axis=0
        )
        w_vT = np.ascontiguousarray(wv_cols.T.astype(BF))
        per_core.append({
            "x": x_bf,
            "w_qkT": w_qkT,
            "w_vT": w_vT,
            "w_outT": w_outT,
            "b_out4": b_out4,
            "e16": e16_bf,
        })
    return per_core


def _prepare_in_maps(inputs):
    """v4 fast path: per-core exp'd, window-expanded bias tables."""
    table = np.asarray(inputs["bias_table"], np.float32)
    c_table = np.exp(table.astype(np.float64)).astype(np.float32)  # [3969, 16]
    # window expansion: idx[kx, u, qx] = 63*u + qx + 31 - kx  (all in range)
    kx = np.arange(32)[:, None, None]
    u = np.arange(63)[None, :, None]
    qx = np.arange(32)[None, None, :]
    widx = (63 * u + qx + 31 - kx).reshape(-1)  # [32*63*32]
    in_maps = _common_prep(inputs)
    for c in range(NCORES):
        be = np.stack([c_table[widx, 2 * c], c_table[widx, 2 * c + 1]])
        be = np.tile(be[:, None, :], (1, 4, 1)).reshape(2, 258048)
        in_maps[c]["bias_e"] = np.ascontiguousarray(be.astype(BF))
    return in_maps


def _prepare_in_maps_gather(inputs):
    """v3 fallback: raw table + i16 gather indices."""
    table = np.asarray(inputs["bias_table"], np.float32)
    ridx = np.asarray(inputs["rel_index"]).astype(np.int64).reshape(N, N)
    tab_rep = np.ascontiguousarray(table.T[np.arange(128) % HEADS].astype(np.float32))
    in_maps = _common_prep(inputs)
    for c in range(NCORES):
        sl = ridx[:, KCH * c:KCH * (c + 1)]
        idxw = np.empty((128, N), np.int16)
        for g in range(8):
            arr = sl[:, 16 * g:16 * (g + 1)].T.reshape(-1)
            idxw[16 * g:16 * (g + 1)] = arr.reshape(N, 16).T
        in_maps[c]["table"] = tab_rep
        in_maps[c]["idx"] = np.ascontiguousarray(idxw)
    return in_maps


def _get_graph(**kw):
    key = ("v4",) + tuple(sorted(kw.items()))
    if key not in _GRAPH_CACHE:
        _GRAPH_CACHE[key] = _build_graph(**kw)
    return _GRAPH_CACHE[key]


def _get_graph_gather(**kw):
    key = ("v3",) + tuple(sorted(kw.items()))
    if key not in _GRAPH_CACHE:
        _GRAPH_CACHE[key] = _build_graph_gather(**kw)
    return _GRAPH_CACHE[key]


def run_on_hw(inputs, trace=False, **kw):
    from concourse.bass_utils import run_bass_kernel_spmd

    nc = _get_graph()
    in_maps = _prepare_in_maps(inputs)
    return run_bass_kernel_spmd(nc, in_maps, core_ids=list(range(NCORES)), trace=trace, **kw)


def run_on_hw_gather(inputs, trace=False, **kw):
    from concourse.bass_utils import run_bass_kernel_spmd

    nc = _get_graph_gather()
    in_maps = _prepare_in_maps_gather(inputs)
    return run_bass_kernel_spmd(nc, in_maps, core_ids=list(range(NCORES)), trace=trace, **kw)


def kernel(**inputs) -> np.ndarray:
    ridx = np.asarray(inputs["rel_index"]).ravel()
    if ridx.shape == (N * N,) and np.array_equal(ridx, _toeplitz_rel_index()):
        res = run_on_hw(inputs).results
    else:
        res = run_on_hw_gather(inputs).results
    out = np.stack([np.asarray(res[c]["out"], np.float32) for c in range(NCORES)])
    return out.reshape(B, OUP, 32, 32)


if __name__ == "__main__":
    _get_graph()
    print("graph built + compiled OK")


# revision 18
# speedup vs baseline: 1.3026x; 1.0399x over previous
"""Trainium2 Bass kernel for nn_Attention_14955076125505.

Windowed self-attention with relative-position bias:
  x:(8,512,32,32) -> qkv -> 16-head attention(N=1024, d=32) + bias_table[rel_index]
  -> out proj -> (8,512,32,32)

Sharding (8 NeuronCores), v4:
  - tensor-parallel over heads: core c owns heads (2c, 2c+1) for qkv + attention.
  - bias: rel_index from the reference is the deterministic 2D relative-
    position (Toeplitz) layout: idx(q,k) = 63*(qy-ky+31) + (qx-kx+31).
    kernel() verifies this host-side; when it holds, each per-(kb) bias tile
    c[key=128kb+p, head e, q] = exp(table[idx, h]) is a strided window read
    over the 63x63 exp'd table, fetched by ONE DMA per kb with a
    hand-built access pattern (negative strides on the key dims).  This
    replaces the previous GpSimd ap_gather + fp8 + AllToAll frontend
    (~300us/rep).  If rel_index ever fails the check, a gather-based
    fallback graph (v3) is used instead.
  - attention output is shipped UNNORMALIZED (+ the per-(head,q) softmax
    denominator as a 33rd row) through an AllToAll that re-shards by
    batch; the batch owner computes reciprocals, broadcasts them over d via a
    small matmul, normalizes, and runs the output projection for its batch.
Compute dtype bf16 on the TensorEngine (f32 PSUM accumulation), exp on ScalarE,
bias multiply on VectorE.
"""

import sys

if "/opt/trn_rl_repo" not in sys.path:
    sys.path.insert(0, "/opt/trn_rl_repo")

import numpy as np
import ml_dtypes

B = 8
C = 512
N = 1024  # H*W
HEADS = 16
D = 32
OUP = 512
TABLE = 3969
NCORES = 8
HPC = 2  # heads per core
KCH = 128  # keys gathered per core (fallback path)
SCALE = D ** -0.5

BF = ml_dtypes.bfloat16

_GRAPH_CACHE = {}

# owner-side reciprocal broadcast selectors, one per kc pair of source cores:
# row r of ao_sb[:, kc, :] is (core 2kc + r//64, head-in-core (r%64)//32, d),
# i.e. global head 2*(2kc + r//64) + (r%64)//32; E16[kc][head, r] = 1 selects
# that head's reciprocal row.
E16 = np.zeros((16, 4, 128), np.float32)
for _kc in range(4):
    for _r in range(128):
        _head = 2 * (2 * _kc + _r // 64) + (_r % 64) // 32
        E16[_head, _kc, _r] = 1.0


def _toeplitz_rel_index():
    yy, xx = np.meshgrid(np.arange(32), np.arange(32), indexing="ij")
    coords = np.stack([yy.ravel(), xx.ravel()])
    rel = coords[:, :, None] - coords[:, None, :]
    rel[0] += 31
    rel[1] += 31
    rel[0] *= 63
    return rel.sum(0).ravel()  # (N*N,) q-major


# ============================================================================
# v4 graph: DMA-window bias (Toeplitz fast path)
# ============================================================================

def _build_graph(repeat=1, collectives=True, num_devices=NCORES, skip_bias=False,
                 skip_mult=False, at_bufs=4, st_bufs=1, split_st=False,
                 mult_pool=0):
    import concourse.bass as bass
    import concourse.mybir as mybir
    import concourse.tile as tile
    from concourse import bacc

    fp32 = mybir.dt.float32
    bf16 = mybir.dt.bfloat16
    stb = 2 if split_st else st_bufs
    stpad = [128, N if split_st else 2 * N]

    nc = bacc.Bacc(
        "TRN2",
        target_bir_lowering=False,
        debug=False,
        enable_asserts=True,
        num_devices=num_devices,
    )

    # ---- kernel I/O (per-core shards, prepared host-side) ----
    x_d = nc.dram_tensor("x", [B, C, N], bf16, kind="ExternalInput").ap()
    wqk_d = nc.dram_tensor("w_qkT", [C, 128], bf16, kind="ExternalInput").ap()
    wv_d = nc.dram_tensor("w_vT", [C, 2 * D], bf16, kind="ExternalInput").ap()
    wo_d = nc.dram_tensor("w_outT", [OUP, OUP], bf16, kind="ExternalInput").ap()
    bo_d = nc.dram_tensor("b_out4", [128, 4], fp32, kind="ExternalInput").ap()
    e16_d = nc.dram_tensor("e16", [16, 4, 128], bf16, kind="ExternalInput").ap()
    # exp'd bias table for this core's 2 heads, window-expanded and
    # replicated 4x over ky (see _prepare_in_maps):
    # bias_e[e, 64512*ky + 2016*kx + 32*u + qx] = exp(table[63*u + qx +
    # 31 - kx, 2c+e]); per (kb, e) the c tile is then a 3-dim all-positive
    # strided DMA with 1024-element contiguous runs.
    be_d = nc.dram_tensor("bias_e", [2, 258048], bf16, kind="ExternalInput").ap()
    out_d = nc.dram_tensor("out", [OUP, N], fp32, kind="ExternalOutput").ap()

    # attention-out a2a: 66 rows = 2 heads x (32 out + 1 denominator)
    ao_in = nc.dram_tensor("ao_a2a_in", [NCORES, 2 * (D + 1), N], bf16).ap()
    ao_out = nc.dram_tensor("ao_a2a_out", [NCORES, 2 * (D + 1), N], bf16).ap()

    RG = [list(range(NCORES))]

    def bias_window_ap(kb, e):
        """Source AP over bias_e for the c tile of key-block kb, head e:
        element (p=(ky',kx), qy, qx) reads the exp'd bias for
        key=(4*kb+ky', kx), query=(qy, qx): addr = e*258048 + 64512*ky' +
        2016*kx + 32*(qy + 31 - 4*kb - ky') + qx, i.e. ky' stride
        64512-32 = 64480 against the ky-replicated table."""
        return bass.AP(
            tensor=be_d.tensor,
            offset=e * 258048 + (31 - 4 * kb) * 32,
            ap=[[64480, 4], [2016, 32], [1, 1024]],
        )

    with tile.TileContext(nc) as tc:
        with (
            tc.tile_pool(name="const", bufs=1) as cp,
            tc.tile_pool(name="persist", bufs=1) as pp,
            tc.tile_pool(name="work", bufs=3) as wp,
            tc.tile_pool(name="psum", bufs=2, space="PSUM") as psp,
            tc.tile_pool(name="psav", bufs=2, space="PSUM") as pav,
        ):
            # ================= constants =================
            wqk_sb = cp.tile([128, 4, 128], bf16)
            nc.sync.dma_start(wqk_sb[:, :, :], wqk_d.rearrange("(kc p) m -> p kc m", p=128))
            wv_sb = cp.tile([128, 4, 2 * D], bf16)
            nc.sync.dma_start(wv_sb[:, :, :], wv_d.rearrange("(kc p) m -> p kc m", p=128))
            wo_sb = cp.tile([128, 4, OUP], bf16)
            nc.sync.dma_start(wo_sb[:, :, :], wo_d.rearrange("(kc p) m -> p kc m", p=128))
            bo_sb = cp.tile([128, 4], fp32)
            nc.sync.dma_start(bo_sb[:, :], bo_d)
            e16_sb = cp.tile([16, 4, 128], bf16)
            nc.sync.dma_start(e16_sb[:, :, :], e16_d)

            for _rep in range(repeat):
                # bias tiles for all 8 key-blocks (shared across batches)
                c_sb = pp.tile([128, 8, HPC * N], bf16, name="c_sb")
                if skip_bias:
                    nc.vector.memset(c_sb[:, :, :], 1.0)
                else:
                    for kb in range(8):
                        for e in range(HPC):
                            nc.sync.dma_start(c_sb[:, kb, e * N:(e + 1) * N],
                                              bias_window_ap(kb, e))

                # ================= qkv projection (all 8 batches) =================
                q_sb = pp.tile([64, B, N], bf16, name="q_sb")
                k_sb = pp.tile([64, B, N], bf16, name="k_sb")
                v_sb = pp.tile([128, B, 8, HPC, D + 1], bf16, name="v_sb")
                nc.vector.memset(v_sb[:, :, :, :, D], 1.0)

                for b in range(B):
                    x_t = wp.tile([128, 4, N], bf16, tag="xt", bufs=2)
                    nc.sync.dma_start(x_t[:, :, :], x_d[b].rearrange("(kc p) n -> p kc n", p=128))

                    qk_ps = psp.tile([128, N], fp32, tag="st", bufs=stb,
                                     padded_shape=stpad)
                    for kc in range(4):
                        for half in range(2):
                            sl = slice(512 * half, 512 * (half + 1))
                            nc.tensor.matmul(
                                qk_ps[:, sl],
                                wqk_sb[:, kc, :],
                                x_t[:, kc, sl],
                                start=(kc == 0),
                                stop=(kc == 3),
                            )
                    nc.vector.tensor_copy(q_sb[:, b, :], qk_ps[0:64, :])
                    nc.vector.tensor_copy(k_sb[:, b, :], qk_ps[64:128, :])

                    for tbq in range(2):
                        v_ps = psp.tile([128, 4, 2 * D], fp32, tag="st", bufs=stb,
                                        padded_shape=[128, 4, (N if split_st else 2 * N) // 4])
                        for tb4 in range(4):
                            tb = 4 * tbq + tb4
                            for kc in range(4):
                                nc.tensor.matmul(
                                    v_ps[:, tb4, :],
                                    x_t[:, kc, 128 * tb:128 * (tb + 1)],
                                    wv_sb[:, kc, :],
                                    start=(kc == 0),
                                    stop=(kc == 3),
                                )
                        nc.vector.tensor_copy(
                            v_sb[:, b, 4 * tbq:4 * (tbq + 1), :, 0:D],
                            v_ps[:, :, :].rearrange("p t (h d) -> p t h d", h=HPC),
                        )

                # ================= attention =================
                # attnout_sb rows 64h+d: unnormalized out (d<32) + denominator
                # (d=32); head blocks start at partitions 0/64 (32-aligned)
                attnout_sb = pp.tile([128, B, N], bf16, name="attnout_sb")

                for b in range(B):
                    avh = [pav.tile([D + 1, N], fp32, tag=f"av{h}", bufs=1,
                                    padded_shape=[128, N], name=f"av{h}")
                           for h in range(HPC)]
                    for kb in range(8):
                        if split_st:
                            # fine tiles: per-head scores, 2-buf PE<->Act ping-pong
                            for h in range(HPC):
                                st = psp.tile([128, N], fp32, tag="st", bufs=stb,
                                              padded_shape=stpad)
                                for half in range(2):
                                    sl = slice(512 * half, 512 * (half + 1))
                                    nc.tensor.matmul(
                                        st[:, sl],
                                        k_sb[32 * h:32 * (h + 1), b, 128 * kb:128 * (kb + 1)],
                                        q_sb[32 * h:32 * (h + 1), b, sl],
                                        start=True,
                                        stop=True,
                                    )
                                at = wp.tile([128, N], bf16, tag="attn", bufs=at_bufs)
                                nc.scalar.activation(at[:, :], st[:, :], mybir.ActivationFunctionType.Exp)
                                if not skip_mult:
                                    eng = nc.gpsimd if (mult_pool and h == 1) else nc.vector
                                    eng.tensor_mul(at[:, :], at[:, :],
                                                   c_sb[:, kb, h * N:(h + 1) * N])
                                for half in range(2):
                                    sl = slice(512 * half, 512 * (half + 1))
                                    nc.tensor.matmul(
                                        avh[h][:, sl],
                                        v_sb[:, b, kb, h, :],
                                        at[:, sl],
                                        start=(kb == 0),
                                        stop=(kb == 7),
                                    )
                            continue
                        st2 = psp.tile([128, 2 * N], fp32, tag="st", bufs=stb)
                        for h in range(HPC):
                            for half in range(2):
                                sl = slice(512 * half, 512 * (half + 1))
                                nc.tensor.matmul(
                                    st2[:, h * N + 512 * half:h * N + 512 * (half + 1)],
                                    k_sb[32 * h:32 * (h + 1), b, 128 * kb:128 * (kb + 1)],
                                    q_sb[32 * h:32 * (h + 1), b, sl],
                                    start=True,
                                    stop=True,
                                )
                        at = wp.tile([128, 2 * N], bf16, tag="attn", bufs=at_bufs)
                        nc.scalar.activation(at[:, :], st2[:, :], mybir.ActivationFunctionType.Exp)
                        if not skip_mult:
                            if mult_pool:
                                nc.vector.tensor_mul(at[:, 0:N], at[:, 0:N], c_sb[:, kb, 0:N])
                                nc.gpsimd.tensor_mul(at[:, N:], at[:, N:], c_sb[:, kb, N:])
                            else:
                                nc.vector.tensor_mul(at[:, :], at[:, :], c_sb[:, kb, :])
                        for h in range(HPC):
                            for half in range(2):
                                sl = slice(512 * half, 512 * (half + 1))
                                nc.tensor.matmul(
                                    avh[h][:, sl],
                                    v_sb[:, b, kb, h, :],
                                    at[:, h * N + 512 * half:h * N + 512 * (half + 1)],
                                    start=(kb == 0),
                                    stop=(kb == 7),
                                )
                    for h in range(HPC):
                        nc.vector.tensor_copy(
                            attnout_sb[64 * h:64 * h + D + 1, b, :],
                            avh[h][:, :],
                        )
                    nc.sync.dma_start(ao_in[b][0:D + 1], attnout_sb[0:D + 1, b, :])
                    nc.sync.dma_start(ao_in[b][D + 1:], attnout_sb[64:64 + D + 1, b, :])

                # ================= all-to-all: heads -> batch =================
                if collectives:
                    nc.gpsimd.collective_compute(
                        "AllToAll",
                        mybir.AluOpType.bypass,
                        replica_groups=RG,
                        ins=[ao_in.opt()],
                        outs=[ao_out.opt()],
                    )
                else:
                    nc.gpsimd.dma_start(ao_out.opt(), ao_in.opt())

                # ================= normalize + output projection (my batch) =======
                den_sb = wp.tile([16, N], bf16, tag="den", bufs=1)
                nc.sync.dma_start(
                    den_sb[:, :],
                    ao_out.rearrange("s (h x) n -> (s h) x n", x=D + 1)[:, D, :],
                )
                rec_sb = wp.tile([16, N], bf16, tag="rec", bufs=1)
                with nc.allow_low_precision(reason="bf16 softmax denominators are within tolerance"):
                    nc.vector.reciprocal(rec_sb[:, :], den_sb[:, :])

                ao_sb = pp.tile([128, 4, N], bf16, name="ao_sb")
                for kc in range(4):
                    for j in range(2):
                        # both heads in one DMA: src rows (33h+d, d<32), dst
                        # partitions 64j..64j+64
                        nc.sync.dma_start(
                            ao_sb[64 * j:64 * j + 64, kc, :],
                            ao_out[2 * kc + j].rearrange(
                                "(h x) n -> h x n", x=D + 1)[:, 0:D, :],
                        )
                for kc in range(4):
                    bc_ps = pav.tile([128, N], fp32, tag="av0", bufs=1)
                    for half in range(2):
                        sl = slice(512 * half, 512 * (half + 1))
                        nc.tensor.matmul(bc_ps[:, sl], e16_sb[:, kc, :], rec_sb[:, sl],
                                         start=True, stop=True)
                    nc.vector.tensor_mul(ao_sb[:, kc, :], ao_sb[:, kc, :], bc_ps[:, :])

                for mb in range(4):
                    o_ps = psp.tile([128, N], fp32, tag="st", bufs=stb,
                                    padded_shape=stpad)
                    for kc in range(4):
                        for half in range(2):
                            sl = slice(512 * half, 512 * (half + 1))
                            nc.tensor.matmul(
                                o_ps[:, sl],
                                wo_sb[:, kc, 128 * mb:128 * (mb + 1)],
                                ao_sb[:, kc, sl],
                                start=(kc == 0),
                                stop=(kc == 3),
                            )
                    o_sb = wp.tile([128, N], fp32, tag="osb", bufs=2)
                    nc.vector.tensor_scalar_add(o_sb[:, :], o_ps[:, :], bo_sb[:, mb:mb + 1])
                    nc.sync.dma_start(out_d[128 * mb:128 * (mb + 1), :], o_sb[:, :])

    nc.compile()
    return nc


# ============================================================================
# v3 fallback graph: GpSimd gather + fp8 + AllToAll bias frontend
# ============================================================================

def _build_graph_gather(repeat=1, collectives=True, num_devices=NCORES, skip_bias=False,
                        skip_mult=False, fp8_bias=True, at_bufs=4, st_bufs=1,
                        split_st=False, skip_gather=False, nchunk=4):
    import concourse.bass as bass
    import concourse.mybir as mybir
    import concourse.tile as tile
    from concourse import bacc

    fp32 = mybir.dt.float32
    bf16 = mybir.dt.bfloat16
    f8 = mybir.dt.float8e4
    i16 = mybir.dt.int16
    bias_dt = f8 if fp8_bias else bf16
    stb = 2 if split_st else st_bufs
    stpad = [128, N if split_st else 2 * N]

    nc = bacc.Bacc(
        "TRN2",
        target_bir_lowering=False,
        debug=False,
        enable_asserts=True,
        num_devices=num_devices,
    )

    x_d = nc.dram_tensor("x", [B, C, N], bf16, kind="ExternalInput").ap()
    wqk_d = nc.dram_tensor("w_qkT", [C, 128], bf16, kind="ExternalInput").ap()
    wv_d = nc.dram_tensor("w_vT", [C, 2 * D], bf16, kind="ExternalInput").ap()
    wo_d = nc.dram_tensor("w_outT", [OUP, OUP], bf16, kind="ExternalInput").ap()
    bo_d = nc.dram_tensor("b_out4", [128, 4], fp32, kind="ExternalInput").ap()
    tab_d = nc.dram_tensor("table", [128, TABLE], fp32, kind="ExternalInput").ap()
    idx_d = nc.dram_tensor("idx", [128, N], i16, kind="ExternalInput").ap()
    e16_d = nc.dram_tensor("e16", [16, 4, 128], bf16, kind="ExternalInput").ap()
    out_d = nc.dram_tensor("out", [OUP, N], fp32, kind="ExternalOutput").ap()

    ebc_in = nc.dram_tensor("ebc_a2a_in", [16, 8, 16384], bias_dt).ap()
    ebc_out = nc.dram_tensor("ebc_a2a_out", [NCORES, HPC, 8, 16384], bias_dt).ap()
    ao_in = nc.dram_tensor("ao_a2a_in", [NCORES, 2 * (D + 1), N], bf16).ap()
    ao_out = nc.dram_tensor("ao_a2a_out", [NCORES, 2 * (D + 1), N], bf16).ap()

    RG = [list(range(NCORES))]

    with tile.TileContext(nc) as tc:
        with (
            tc.tile_pool(name="const", bufs=1) as cp,
            tc.tile_pool(name="persist", bufs=1) as pp,
            tc.tile_pool(name="work", bufs=3) as wp,
            tc.tile_pool(name="gather", bufs=1) as gp,
            tc.tile_pool(name="psum", bufs=2, space="PSUM") as psp,
            tc.tile_pool(name="psav", bufs=2, space="PSUM") as pav,
        ):
            tab_sb = cp.tile([128, TABLE], fp32)
            nc.sync.dma_start(tab_sb[:, :], tab_d)
            idx_sb = cp.tile([128, N], i16)
            nc.sync.dma_start(idx_sb[:, :], idx_d)
            wqk_sb = cp.tile([128, 4, 128], bf16)
            nc.sync.dma_start(wqk_sb[:, :, :], wqk_d.rearrange("(kc p) m -> p kc m", p=128))
            wv_sb = cp.tile([128, 4, 2 * D], bf16)
            nc.sync.dma_start(wv_sb[:, :, :], wv_d.rearrange("(kc p) m -> p kc m", p=128))
            wo_sb = cp.tile([128, 4, OUP], bf16)
            nc.sync.dma_start(wo_sb[:, :, :], wo_d.rearrange("(kc p) m -> p kc m", p=128))
            bo_sb = cp.tile([128, 4], fp32)
            nc.sync.dma_start(bo_sb[:, :], bo_d)
            e16_sb = cp.tile([16, 4, 128], bf16)
            nc.sync.dma_start(e16_sb[:, :, :], e16_d)

            b8full = cp.tile([128, 8, HPC, N], bias_dt)

            NIDX = 16 * N
            NCHUNK = nchunk
            CH = NIDX // NCHUNK

            def emit_frontend():
                for ch in range(NCHUNK):
                    gath = gp.tile([128, CH], fp32, tag="gath", bufs=1)
                    if skip_gather:
                        nc.gpsimd.memset(gath[:, :], 0.01)
                    else:
                        nc.gpsimd.ap_gather(
                            out_ap=gath[:, :],
                            in_ap=tab_sb[:, :],
                            idxs_ap=idx_sb[:, ch * (CH // 16):(ch + 1) * (CH // 16)],
                            channels=128,
                            num_elems=TABLE,
                            d=1,
                            num_idxs=CH,
                        )
                    b8 = gp.tile([128, CH], bias_dt, tag="b8", bufs=1)
                    nc.gpsimd.tensor_copy(b8[:, :], gath[:, :])
                    nc.gpsimd.dma_start(
                        ebc_in.rearrange("r g (ch f) -> g r ch f", f=CH)[:, :, ch, :],
                        b8[:, :],
                    )
                if collectives:
                    nc.gpsimd.collective_compute(
                        "AllToAll",
                        mybir.AluOpType.bypass,
                        replica_groups=RG,
                        ins=[ebc_in.opt()],
                        outs=[ebc_out.opt()],
                    )
                else:
                    nc.gpsimd.dma_start(
                        ebc_out.rearrange("s e g f -> (s e) g f"), ebc_in.opt())

            def emit_loads():
                for kb in range(8):
                    for e in range(HPC):
                        nc.gpsimd.dma_start(
                            b8full[:, kb, e, :],
                            ebc_out[kb, e].rearrange("g (fq q) -> (g fq) q", q=N),
                        )

            for _rep in range(repeat):
                c_sb = pp.tile([128, 8, HPC * N], bf16, name="c_sb")
                if skip_bias:
                    nc.vector.memset(c_sb[:, :, :], 1.0)
                else:
                    if _rep == 0:
                        emit_frontend()
                        emit_loads()
                    for kb in range(8):
                        for e in range(HPC):
                            nc.vector.tensor_scalar_add(
                                c_sb[:, kb, e * N:(e + 1) * N], b8full[:, kb, e, :], 1.0,
                            )
                q_sb = pp.tile([64, B, N], bf16, name="q_sb")
                k_sb = pp.tile([64, B, N], bf16, name="k_sb")
                v_sb = pp.tile([128, B, 8, HPC, D + 1], bf16, name="v_sb")
                nc.vector.memset(v_sb[:, :, :, :, D], 1.0)

                for b in range(B):
                    x_t = wp.tile([128, 4, N], bf16, tag="xt", bufs=2)
                    nc.sync.dma_start(x_t[:, :, :], x_d[b].rearrange("(kc p) n -> p kc n", p=128))

                    qk_ps = psp.tile([128, N], fp32, tag="st", bufs=stb,
                                     padded_shape=stpad)
                    for kc in range(4):
                        for half in range(2):
                            sl = slice(512 * half, 512 * (half + 1))
                            nc.tensor.matmul(
                                qk_ps[:, sl],
                                wqk_sb[:, kc, :],
                                x_t[:, kc, sl],
                                start=(kc == 0),
                                stop=(kc == 3),
                            )
                    nc.vector.tensor_copy(q_sb[:, b, :], qk_ps[0:64, :])
                    nc.vector.tensor_copy(k_sb[:, b, :], qk_ps[64:128, :])

                    for tbq in range(2):
                        v_ps = psp.tile([128, 4, 2 * D], fp32, tag="st", bufs=stb,
                                        padded_shape=[128, 4, (N if split_st else 2 * N) // 4])
                        for tb4 in range(4):
                            tb = 4 * tbq + tb4
                            for kc in range(4):
                                nc.tensor.matmul(
                                    v_ps[:, tb4, :],
                                    x_t[:, kc, 128 * tb:128 * (tb + 1)],
                                    wv_sb[:, kc, :],
                                    start=(kc == 0),
                                    stop=(kc == 3),
                                )
                        nc.vector.tensor_copy(
                            v_sb[:, b, 4 * tbq:4 * (tbq + 1), :, 0:D],
                            v_ps[:, :, :].rearrange("p t (h d) -> p t h d", h=HPC),
                        )

                attnout_sb = pp.tile([128, B, N], bf16, name="attnout_sb")

                for b in range(B):
                    avh = [pav.tile([D + 1, N], fp32, tag=f"av{h}", bufs=1,
                                    padded_shape=[128, N], name=f"av{h}")
                           for h in range(HPC)]
                    for kb in range(8):
                        if split_st:
                            for h in range(HPC):
                                st = psp.tile([128, N], fp32, tag="st", bufs=stb,
                                              padded_shape=stpad)
                                for half in range(2):
                                    sl = slice(512 * half, 512 * (half + 1))
                                    nc.tensor.matmul(
                                        st[:, sl],
                                        k_sb[32 * h:32 * (h + 1), b, 128 * kb:128 * (kb + 1)],
                                        q_sb[32 * h:32 * (h + 1), b, sl],
                                        start=True,
                                        stop=True,
                                    )
                                at = wp.tile([128, N], bf16, tag="attn", bufs=at_bufs)
                                nc.scalar.activation(at[:, :], st[:, :], mybir.ActivationFunctionType.Exp)
                                if not skip_mult:
                                    nc.vector.tensor_mul(at[:, :], at[:, :],
                                                         c_sb[:, kb, h * N:(h + 1) * N])
                                for half in range(2):
                                    sl = slice(512 * half, 512 * (half + 1))
                                    nc.tensor.matmul(
                                        avh[h][:, sl],
                                        v_sb[:, b, kb, h, :],
                                        at[:, sl],
                                        start=(kb == 0),
                                        stop=(kb == 7),
                                    )
                            continue
                        st2 = psp.tile([128, 2 * N], fp32, tag="st", bufs=stb)
                        for h in range(HPC):
                            for half in range(2):
                                sl = slice(512 * half, 512 * (half + 1))
                                nc.tensor.matmul(
                                    st2[:, h * N + 512 * half:h * N + 512 * (half + 1)],
                                    k_sb[32 * h:32 * (h + 1), b, 128 * kb:128 * (kb + 1)],
                                    q_sb[32 * h:32 * (h + 1), b, sl],
                                    start=True,
                                    stop=True,
                                )
                        at = wp.tile([128, 2 * N], bf16, tag="attn", bufs=at_bufs)
                        nc.scalar.activation(at[:, :], st2[:, :], mybir.ActivationFunctionType.Exp)
                        if not skip_mult:
                            nc.vector.tensor_mul(at[:, :], at[:, :], c_sb[:, kb, :])
                        for h in range(HPC):
                            for half in range(2):
                                sl = slice(512 * half, 512 * (half + 1))
                                nc.tensor.matmul(
                                    avh[h][:, sl],
                                    v_sb[:, b, kb, h, :],
                                    at[:, h * N + 512 * half:h * N + 512 * (half + 1)],
                                    start=(kb == 0),
                                    stop=(kb == 7),
                                )
                    for h in range(HPC):
                        nc.vector.tensor_copy(
                            attnout_sb[64 * h:64 * h + D + 1, b, :],
                            avh[h][:, :],
                        )
                    nc.sync.dma_start(ao_in[b][0:D + 1], attnout_sb[0:D + 1, b, :])
                    nc.sync.dma_start(ao_in[b][D + 1:], attnout_sb[64:64 + D + 1, b, :])

                if not skip_bias and _rep + 1 < repeat:
                    emit_frontend()

                if collectives:
                    nc.gpsimd.collective_compute(
                        "AllToAll",
                        mybir.AluOpType.bypass,
                        replica_groups=RG,
                        ins=[ao_in.opt()],
                        outs=[ao_out.opt()],
                    )
                else:
                    nc.gpsimd.dma_start(ao_out.opt(), ao_in.opt())
                if not skip_bias and _rep + 1 < repeat:
                    emit_loads()

                den_sb = wp.tile([16, N], bf16, tag="den", bufs=1)
                nc.sync.dma_start(
                    den_sb[:, :],
                    ao_out.rearrange("s (h x) n -> (s h) x n", x=D + 1)[:, D, :],
                )
                rec_sb = wp.tile([16, N], bf16, tag="rec", bufs=1)
                with nc.allow_low_precision(reason="bf16 softmax denominators are within tolerance"):
                    nc.vector.reciprocal(rec_sb[:, :], den_sb[:, :])

                ao_sb = pp.tile([128, 4, N], bf16, name="ao_sb")
                for kc in range(4):
                    for j in range(2):
                        for h in range(HPC):
                            nc.sync.dma_start(
                                ao_sb[64 * j + 32 * h:64 * j + 32 * h + D, kc, :],
                                ao_out[2 * kc + j, (D + 1) * h:(D + 1) * h + D, :],
                            )
                for kc in range(4):
                    bc_ps = pav.tile([128, N], fp32, tag="av0", bufs=1)
                    for half in range(2):
                        sl = slice(512 * half, 512 * (half + 1))
                        nc.tensor.matmul(bc_ps[:, sl], e16_sb[:, kc, :], rec_sb[:, sl],
                                         start=True, stop=True)
                    nc.vector.tensor_mul(ao_sb[:, kc, :], ao_sb[:, kc, :], bc_ps[:, :])

                for mb in range(4):
                    o_ps = psp.tile([128, N], fp32, tag="st", bufs=stb,
                                    padded_shape=stpad)
                    for kc in range(4):
                        for half in range(2):
                            sl = slice(512 * half, 512 * (half + 1))
                            nc.tensor.matmul(
                                o_ps[:, sl],
                                wo_sb[:, kc, 128 * mb:128 * (mb + 1)],
                                ao_sb[:, kc, sl],
                                start=(kc == 0),
                                stop=(kc == 3),
                            )
                    o_sb = wp.tile([128, N], fp32, tag="osb", bufs=2)
                    nc.vector.tensor_scalar_add(o_sb[:, :], o_ps[:, :], bo_sb[:, mb:mb + 1])
                    nc.sync.dma_start(out_d[128 * mb:128 * (mb + 1), :], o_sb[:, :])

    nc.compile()
    return nc


# ============================================================================
# host-side prep
# ============================================================================

def _common_prep(inputs):
    x = np.asarray(inputs["x"], np.float32).reshape(B, C, N)
    w_qkv = np.asarray(inputs["w_qkv"], np.float32)
    w_out = np.asarray(inputs["w_out"], np.float32)
    b_out = np.asarray(inputs["b_out"], np.float32)

    x_bf = np.ascontiguousarray(x.astype(BF))
    wq = w_qkv[0:OUP]
    wk = w_qkv[OUP:2 * OUP]
    wv = w_qkv[2 * OUP:3 * OUP]
    w_outT = np.ascontiguousarray(w_out.T.astype(BF))
    b_out4 = np.ascontiguousarray(b_out.reshape(4, 128).T.astype(np.float32))
    e16_bf = np.ascontiguousarray(E16.astype(BF))

    per_core = []
    for c in range(NCORES):
        h0, h1 = 2 * c, 2 * c + 1
        wqk_cols = np.concatenate(
            [
                wq[h0 * D:(h0 + 1) * D] * SCALE,
                wq[h1 * D:(h1 + 1) * D] * SCALE,
                wk[h0 * D:(h0 + 1) * D],
                wk[h1 * D:(h1 + 1) * D],
            ],
            axis=0,
        )
        w_qkT = np.ascontiguousarray(wqk_cols.T.astype(BF))
        wv_cols = np.concatenate(
            [wv[h0 * D:(h0 + 1) * D], wv[h1 * D:(h1 + 1) * D]], axis=0
        )
        w_vT = np.ascontiguousarray(wv_cols.T.astype(BF))
        per_core.append({
            "x": x_bf,
            "w_qkT": w_qkT,
            "w_vT": w_vT,
            "w_outT": w_outT,
            "b_out4": b_out4,
            "e16": e16_bf,
        })
    return per_core


def _prepare_in_maps(inputs):
    """v4 fast path: per-core exp'd, window-expanded bias tables."""
    table = np.asarray(inputs["bias_table"], np.float32)
    c_table = np.exp(table.astype(np.float64)).astype(np.float32)  # [3969, 16]
    # window expansion: idx[kx, u, qx] = 63*u + qx + 31 - kx  (all in range)
    kx = np.arange(32)[:, None, None]
    u = np.arange(63)[None, :, None]
    qx = np.arange(32)[None, None, :]
    widx = (63 * u + qx + 31 - kx).reshape(-1)  # [32*63*32]
    in_maps = _common_prep(inputs)
    for c in range(NCORES):
        be = np.stack([c_table[widx, 2 * c], c_table[widx, 2 * c + 1]])
        be = np.tile(be[:, None, :], (1, 4, 1)).reshape(2, 258048)
        in_maps[c]["bias_e"] = np.ascontiguousarray(be.astype(BF))
    return in_maps


def _prepare_in_maps_gather(inputs):
    """v3 fallback: raw table + i16 gather indices."""
    table = np.asarray(inputs["bias_table"], np.float32)
    ridx = np.asarray(inputs["rel_index"]).astype(np.int64).reshape(N, N)
    tab_rep = np.ascontiguousarray(table.T[np.arange(128) % HEADS].astype(np.float32))
    in_maps = _common_prep(inputs)
    for c in range(NCORES):
        sl = ridx[:, KCH * c:KCH * (c + 1)]
        idxw = np.empty((128, N), np.int16)
        for g in range(8):
            arr = sl[:, 16 * g:16 * (g + 1)].T.reshape(-1)
            idxw[16 * g:16 * (g + 1)] = arr.reshape(N, 16).T
        in_maps[c]["table"] = tab_rep
        in_maps[c]["idx"] = np.ascontiguousarray(idxw)
    return in_maps


def _get_graph(**kw):
    key = ("v4",) + tuple(sorted(kw.items()))
    if key not in _GRAPH_CACHE:
        _GRAPH_CACHE[key] = _build_graph(**kw)
    return _GRAPH_CACHE[key]


def _get_graph_gather(**kw):
    key = ("v3",) + tuple(sorted(kw.items()))
    if key not in _GRAPH_CACHE:
        _GRAPH_CACHE[key] = _build_graph_gather(**kw)
    return _GRAPH_CACHE[key]


def run_on_hw(inputs, trace=False, **kw):
    from concourse.bass_utils import run_bass_kernel_spmd

    nc = _get_graph()
    in_maps = _prepare_in_maps(inputs)
    return run_bass_kernel_spmd(nc, in_maps, core_ids=list(range(NCORES)), trace=trace, **kw)


def run_on_hw_gather(inputs, trace=False, **kw):
    from concourse.bass_utils import run_bass_kernel_spmd

    nc = _get_graph_gather()
    in_maps = _prepare_in_maps_gather(inputs)
    return run_bass_kernel_spmd(nc, in_maps, core_ids=list(range(NCORES)), trace=trace, **kw)


def kernel(**inputs) -> np.ndarray:
    ridx = np.asarray(inputs["rel_index"]).ravel()
    if ridx.shape == (N * N,) and np.array_equal(ridx, _toeplitz_rel_index()):
        res = run_on_hw(inputs).results
    else:
        res = run_on_hw_gather(inputs).results
    out = np.stack([np.asarray(res[c]["out"], np.float32) for c in range(NCORES)])
    return out.reshape(B, OUP, 32, 32)


if __name__ == "__main__":
    _get_graph()
    print("graph built + compiled OK")


# revision 20
# speedup vs baseline: 1.3700x; 1.0518x over previous
"""Trainium2 Bass kernel for nn_Attention_14955076125505.

Windowed self-attention with relative-position bias:
  x:(8,512,32,32) -> qkv -> 16-head attention(N=1024, d=32) + bias_table[rel_index]
  -> out proj -> (8,512,32,32)

Sharding (8 NeuronCores), v4:
  - tensor-parallel over heads: core c owns heads (2c, 2c+1) for qkv + attention.
  - bias: rel_index from the reference is the deterministic 2D relative-
    position (Toeplitz) layout: idx(q,k) = 63*(qy-ky+31) + (qx-kx+31).
    kernel() verifies this host-side; when it holds, each per-(kb) bias tile
    c[key=128kb+p, head e, q] = exp(table[idx, h]) is a strided window read
    over the 63x63 exp'd table, fetched by ONE DMA per kb with a
    hand-built access pattern (negative strides on the key dims).  This
    replaces the previous GpSimd ap_gather + fp8 + AllToAll frontend
    (~300us/rep).  If rel_index ever fails the check, a gather-based
    fallback graph (v3) is used instead.
  - attention output is shipped UNNORMALIZED (+ the per-(head,q) softmax
    denominator as a 33rd row) through an AllToAll that re-shards by
    batch; the batch owner computes reciprocals, broadcasts them over d via a
    small matmul, normalizes, and runs the output projection for its batch.
Compute dtype bf16 on the TensorEngine (f32 PSUM accumulation), exp on ScalarE,
bias multiply on VectorE.
"""

import sys

if "/opt/trn_rl_repo" not in sys.path:
    sys.path.insert(0, "/opt/trn_rl_repo")

import numpy as np
import ml_dtypes

B = 8
C = 512
N = 1024  # H*W
HEADS = 16
D = 32
OUP = 512
TABLE = 3969
NCORES = 8
HPC = 2  # heads per core
KCH = 128  # keys gathered per core (fallback path)
SCALE = D ** -0.5

BF = ml_dtypes.bfloat16

_GRAPH_CACHE = {}

# owner-side reciprocal broadcast selectors, one per kc pair of source cores:
# row r of ao_sb[:, kc, :] is (core 2kc + r//64, head-in-core (r%64)//32, d),
# i.e. global head 2*(2kc + r//64) + (r%64)//32; E16[kc][head, r] = 1 selects
# that head's reciprocal row.
E16 = np.zeros((16, 4, 128), np.float32)
for _kc in range(4):
    for _r in range(128):
        _head = 2 * (2 * _kc + _r // 64) + (_r % 64) // 32
        E16[_head, _kc, _r] = 1.0


def _toeplitz_rel_index():
    yy, xx = np.meshgrid(np.arange(32), np.arange(32), indexing="ij")
    coords = np.stack([yy.ravel(), xx.ravel()])
    rel = coords[:, :, None] - coords[:, None, :]
    rel[0] += 31
    rel[1] += 31
    rel[0] *= 63
    return rel.sum(0).ravel()  # (N*N,) q-major


# ============================================================================
# v4 graph: DMA-window bias (Toeplitz fast path)
# ============================================================================

def _build_graph(repeat=1, collectives=True, num_devices=NCORES, skip_bias=False,
                 skip_mult=False, at_bufs=8, st_bufs=1, split_st=False,
                 mult_pool=1, qkv_bufs=1):
    import concourse.bass as bass
    import concourse.mybir as mybir
    import concourse.tile as tile
    from concourse import bacc

    fp32 = mybir.dt.float32
    bf16 = mybir.dt.bfloat16
    stb = 2 if split_st else st_bufs
    stpad = [128, N if split_st else 2 * N]

    nc = bacc.Bacc(
        "TRN2",
        target_bir_lowering=False,
        debug=False,
        enable_asserts=True,
        num_devices=num_devices,
    )

    # ---- kernel I/O (per-core shards, prepared host-side) ----
    x_d = nc.dram_tensor("x", [B, C, N], bf16, kind="ExternalInput").ap()
    wqk_d = nc.dram_tensor("w_qkT", [C, 128], bf16, kind="ExternalInput").ap()
    wv_d = nc.dram_tensor("w_vT", [C, 2 * D], bf16, kind="ExternalInput").ap()
    wo_d = nc.dram_tensor("w_outT", [OUP, OUP], bf16, kind="ExternalInput").ap()
    bo_d = nc.dram_tensor("b_out4", [128, 4], fp32, kind="ExternalInput").ap()
    e16_d = nc.dram_tensor("e16", [16, 4, 128], bf16, kind="ExternalInput").ap()
    # exp'd bias table for this core's 2 heads, window-expanded and
    # replicated 4x over ky (see _prepare_in_maps):
    # bias_e[e, 64512*ky + 2016*kx + 32*u + qx] = exp(table[63*u + qx +
    # 31 - kx, 2c+e]); per (kb, e) the c tile is then a 3-dim all-positive
    # strided DMA with 1024-element contiguous runs.
    be_d = nc.dram_tensor("bias_e", [2, 258048], bf16, kind="ExternalInput").ap()
    out_d = nc.dram_tensor("out", [OUP, N], fp32, kind="ExternalOutput").ap()

    # attention-out a2a: 66 rows = 2 heads x (32 out + 1 denominator)
    ao_in = nc.dram_tensor("ao_a2a_in", [NCORES, 2 * (D + 1), N], bf16).ap()
    ao_out = nc.dram_tensor("ao_a2a_out", [NCORES, 2 * (D + 1), N], bf16).ap()

    RG = [list(range(NCORES))]

    def bias_window_ap(kb, e):
        """Source AP over bias_e for the c tile of key-block kb, head e:
        element (p=(ky',kx), qy, qx) reads the exp'd bias for
        key=(4*kb+ky', kx), query=(qy, qx): addr = e*258048 + 64512*ky' +
        2016*kx + 32*(qy + 31 - 4*kb - ky') + qx, i.e. ky' stride
        64512-32 = 64480 against the ky-replicated table."""
        return bass.AP(
            tensor=be_d.tensor,
            offset=e * 258048 + (31 - 4 * kb) * 32,
            ap=[[64480, 4], [2016, 32], [1, 1024]],
        )

    with tile.TileContext(nc) as tc:
        with (
            tc.tile_pool(name="const", bufs=1) as cp,
            tc.tile_pool(name="persist", bufs=1) as pp,
            tc.tile_pool(name="work", bufs=3) as wp,
            tc.tile_pool(name="psum", bufs=2, space="PSUM") as psp,
            tc.tile_pool(name="psav", bufs=2, space="PSUM") as pav,
        ):
            # ================= constants =================
            wqk_sb = cp.tile([128, 4, 128], bf16)
            nc.sync.dma_start(wqk_sb[:, :, :], wqk_d.rearrange("(kc p) m -> p kc m", p=128))
            wv_sb = cp.tile([128, 4, 2 * D], bf16)
            nc.sync.dma_start(wv_sb[:, :, :], wv_d.rearrange("(kc p) m -> p kc m", p=128))
            wo_sb = cp.tile([128, 4, OUP], bf16)
            nc.sync.dma_start(wo_sb[:, :, :], wo_d.rearrange("(kc p) m -> p kc m", p=128))
            bo_sb = cp.tile([128, 4], fp32)
            nc.sync.dma_start(bo_sb[:, :], bo_d)
            e16_sb = cp.tile([16, 4, 128], bf16)
            nc.sync.dma_start(e16_sb[:, :, :], e16_d)

            for _rep in range(repeat):
                # bias tiles for all 8 key-blocks (shared across batches)
                c_sb = pp.tile([128, 8, HPC * N], bf16, name="c_sb")
                if skip_bias:
                    nc.vector.memset(c_sb[:, :, :], 1.0)
                else:
                    for kb in range(8):
                        for e in range(HPC):
                            nc.sync.dma_start(c_sb[:, kb, e * N:(e + 1) * N],
                                              bias_window_ap(kb, e))

                # ================= qkv projection (all 8 batches) =================
                q_sb = pp.tile([64, B, N], bf16, tag="q_sb", bufs=qkv_bufs)
                k_sb = pp.tile([64, B, N], bf16, tag="k_sb", bufs=qkv_bufs)
                v_sb = pp.tile([128, B, 8, HPC, D + 1], bf16, tag="v_sb", bufs=qkv_bufs)
                nc.vector.memset(v_sb[:, :, :, :, D], 1.0)

                for b in range(B):
                    x_t = wp.tile([128, 4, N], bf16, tag="xt", bufs=2)
                    nc.sync.dma_start(x_t[:, :, :], x_d[b].rearrange("(kc p) n -> p kc n", p=128))

                    qk_ps = psp.tile([128, N], fp32, tag="st", bufs=stb,
                                     padded_shape=stpad)
                    for kc in range(4):
                        for half in range(2):
                            sl = slice(512 * half, 512 * (half + 1))
                            nc.tensor.matmul(
                                qk_ps[:, sl],
                                wqk_sb[:, kc, :],
                                x_t[:, kc, sl],
                                start=(kc == 0),
                                stop=(kc == 3),
                            )
                    nc.vector.tensor_copy(q_sb[:, b, :], qk_ps[0:64, :])
                    nc.vector.tensor_copy(k_sb[:, b, :], qk_ps[64:128, :])

                    for tbq in range(2):
                        v_ps = psp.tile([128, 4, 2 * D], fp32, tag="st", bufs=stb,
                                        padded_shape=[128, 4, (N if split_st else 2 * N) // 4])
                        for tb4 in range(4):
                            tb = 4 * tbq + tb4
                            for kc in range(4):
                                nc.tensor.matmul(
                                    v_ps[:, tb4, :],
                                    x_t[:, kc, 128 * tb:128 * (tb + 1)],
                                    wv_sb[:, kc, :],
                                    start=(kc == 0),
                                    stop=(kc == 3),
                                )
                        nc.vector.tensor_copy(
                            v_sb[:, b, 4 * tbq:4 * (tbq + 1), :, 0:D],
                            v_ps[:, :, :].rearrange("p t (h d) -> p t h d", h=HPC),
                        )

                # ================= attention =================
                # attnout_sb rows 64h+d: unnormalized out (d<32) + denominator
                # (d=32); head blocks start at partitions 0/64 (32-aligned)
                attnout_sb = pp.tile([128, B, N], bf16, name="attnout_sb")

                for b in range(B):
                    avh = [pav.tile([D + 1, N], fp32, tag=f"av{h}", bufs=1,
                                    padded_shape=[128, N], name=f"av{h}")
                           for h in range(HPC)]
                    for kb in range(8):
                        if split_st:
                            # fine tiles: per-head scores, 2-buf PE<->Act ping-pong
                            for h in range(HPC):
                                st = psp.tile([128, N], fp32, tag="st", bufs=stb,
                                              padded_shape=stpad)
                                for half in range(2):
                                    sl = slice(512 * half, 512 * (half + 1))
                                    nc.tensor.matmul(
                                        st[:, sl],
                                        k_sb[32 * h:32 * (h + 1), b, 128 * kb:128 * (kb + 1)],
                                        q_sb[32 * h:32 * (h + 1), b, sl],
                                        start=True,
                                        stop=True,
                                    )
                                at = wp.tile([128, N], bf16, tag="attn", bufs=at_bufs)
                                nc.scalar.activation(at[:, :], st[:, :], mybir.ActivationFunctionType.Exp)
                                if not skip_mult:
                                    eng = nc.gpsimd if (mult_pool and h == 1) else nc.vector
                                    eng.tensor_mul(at[:, :], at[:, :],
                                                   c_sb[:, kb, h * N:(h + 1) * N])
                                for half in range(2):
                                    sl = slice(512 * half, 512 * (half + 1))
                                    nc.tensor.matmul(
                                        avh[h][:, sl],
                                        v_sb[:, b, kb, h, :],
                                        at[:, sl],
                                        start=(kb == 0),
                                        stop=(kb == 7),
                                    )
                            continue
                        st2 = psp.tile([128, 2 * N], fp32, tag="st", bufs=stb)
                        for h in range(HPC):
                            for half in range(2):
                                sl = slice(512 * half, 512 * (half + 1))
                                nc.tensor.matmul(
                                    st2[:, h * N + 512 * half:h * N + 512 * (half + 1)],
                                    k_sb[32 * h:32 * (h + 1), b, 128 * kb:128 * (kb + 1)],
                                    q_sb[32 * h:32 * (h + 1), b, sl],
                                    start=True,
                                    stop=True,
                                )
                        at = wp.tile([128, 2 * N], bf16, tag="attn", bufs=at_bufs)
                        nc.scalar.activation(at[:, :], st2[:, :], mybir.ActivationFunctionType.Exp)
                        if not skip_mult:
                            if mult_pool:
                                nc.vector.tensor_mul(at[:, 0:N], at[:, 0:N], c_sb[:, kb, 0:N])
                                nc.gpsimd.tensor_mul(at[:, N:], at[:, N:], c_sb[:, kb, N:])
                            else:
                                nc.vector.tensor_mul(at[:, :], at[:, :], c_sb[:, kb, :])
                        for h in range(HPC):
                            for half in range(2):
                                sl = slice(512 * half, 512 * (half + 1))
                                nc.tensor.matmul(
                                    avh[h][:, sl],
                                    v_sb[:, b, kb, h, :],
                                    at[:, h * N + 512 * half:h * N + 512 * (half + 1)],
                                    start=(kb == 0),
                                    stop=(kb == 7),
                                )
                    for h in range(HPC):
                        nc.vector.tensor_copy(
                            attnout_sb[64 * h:64 * h + D + 1, b, :],
                            avh[h][:, :],
                        )
                    nc.sync.dma_start(ao_in[b][0:D + 1], attnout_sb[0:D + 1, b, :])
                    nc.sync.dma_start(ao_in[b][D + 1:], attnout_sb[64:64 + D + 1, b, :])

                # ================= all-to-all: heads -> batch =================
                if collectives:
                    nc.gpsimd.collective_compute(
                        "AllToAll",
                        mybir.AluOpType.bypass,
                        replica_groups=RG,
                        ins=[ao_in.opt()],
                        outs=[ao_out.opt()],
                    )
                else:
                    nc.gpsimd.dma_start(ao_out.opt(), ao_in.opt())

                # ================= normalize + output projection (my batch) =======
                den_sb = wp.tile([16, N], bf16, tag="den", bufs=1)
                nc.sync.dma_start(
                    den_sb[:, :],
                    ao_out.rearrange("s (h x) n -> (s h) x n", x=D + 1)[:, D, :],
                )
                rec_sb = wp.tile([16, N], bf16, tag="rec", bufs=1)
                with nc.allow_low_precision(reason="bf16 softmax denominators are within tolerance"):
                    nc.vector.reciprocal(rec_sb[:, :], den_sb[:, :])

                ao_sb = pp.tile([128, 4, N], bf16, name="ao_sb")
                for kc in range(4):
                    for j in range(2):
                        # both heads in one DMA: src rows (33h+d, d<32), dst
                        # partitions 64j..64j+64
                        nc.sync.dma_start(
                            ao_sb[64 * j:64 * j + 64, kc, :],
                            ao_out[2 * kc + j].rearrange(
                                "(h x) n -> h x n", x=D + 1)[:, 0:D, :],
                        )
                for kc in range(4):
                    bc_ps = pav.tile([128, N], fp32, tag="av0", bufs=1)
                    for half in range(2):
                        sl = slice(512 * half, 512 * (half + 1))
                        nc.tensor.matmul(bc_ps[:, sl], e16_sb[:, kc, :], rec_sb[:, sl],
                                         start=True, stop=True)
                    nc.vector.tensor_mul(ao_sb[:, kc, :], ao_sb[:, kc, :], bc_ps[:, :])

                for mb in range(4):
                    o_ps = psp.tile([128, N], fp32, tag="st", bufs=stb,
                                    padded_shape=stpad)
                    for kc in range(4):
                        for half in range(2):
                            sl = slice(512 * half, 512 * (half + 1))
                            nc.tensor.matmul(
                                o_ps[:, sl],
                                wo_sb[:, kc, 128 * mb:128 * (mb + 1)],
                                ao_sb[:, kc, sl],
                                start=(kc == 0),
                                stop=(kc == 3),
                            )
                    o_sb = wp.tile([128, N], fp32, tag="osb", bufs=2)
                    nc.vector.tensor_scalar_add(o_sb[:, :], o_ps[:, :], bo_sb[:, mb:mb + 1])
                    nc.sync.dma_start(out_d[128 * mb:128 * (mb + 1), :], o_sb[:, :])

    nc.compile()
    return nc


# ============================================================================
# v3 fallback graph: GpSimd gather + fp8 + AllToAll bias frontend
# ============================================================================

def _build_graph_gather(repeat=1, collectives=True, num_devices=NCORES, skip_bias=False,
                        skip_mult=False, fp8_bias=True, at_bufs=4, st_bufs=1,
                        split_st=False, skip_gather=False, nchunk=4):
    import concourse.bass as bass
    import concourse.mybir as mybir
    import concourse.tile as tile
    from concourse import bacc

    fp32 = mybir.dt.float32
    bf16 = mybir.dt.bfloat16
    f8 = mybir.dt.float8e4
    i16 = mybir.dt.int16
    bias_dt = f8 if fp8_bias else bf16
    stb = 2 if split_st else st_bufs
    stpad = [128, N if split_st else 2 * N]

    nc = bacc.Bacc(
        "TRN2",
        target_bir_lowering=False,
        debug=False,
        enable_asserts=True,
        num_devices=num_devices,
    )

    x_d = nc.dram_tensor("x", [B, C, N], bf16, kind="ExternalInput").ap()
    wqk_d = nc.dram_tensor("w_qkT", [C, 128], bf16, kind="ExternalInput").ap()
    wv_d = nc.dram_tensor("w_vT", [C, 2 * D], bf16, kind="ExternalInput").ap()
    wo_d = nc.dram_tensor("w_outT", [OUP, OUP], bf16, kind="ExternalInput").ap()
    bo_d = nc.dram_tensor("b_out4", [128, 4], fp32, kind="ExternalInput").ap()
    tab_d = nc.dram_tensor("table", [128, TABLE], fp32, kind="ExternalInput").ap()
    idx_d = nc.dram_tensor("idx", [128, N], i16, kind="ExternalInput").ap()
    e16_d = nc.dram_tensor("e16", [16, 4, 128], bf16, kind="ExternalInput").ap()
    out_d = nc.dram_tensor("out", [OUP, N], fp32, kind="ExternalOutput").ap()

    ebc_in = nc.dram_tensor("ebc_a2a_in", [16, 8, 16384], bias_dt).ap()
    ebc_out = nc.dram_tensor("ebc_a2a_out", [NCORES, HPC, 8, 16384], bias_dt).ap()
    ao_in = nc.dram_tensor("ao_a2a_in", [NCORES, 2 * (D + 1), N], bf16).ap()
    ao_out = nc.dram_tensor("ao_a2a_out", [NCORES, 2 * (D + 1), N], bf16).ap()

    RG = [list(range(NCORES))]

    with tile.TileContext(nc) as tc:
        with (
            tc.tile_pool(name="const", bufs=1) as cp,
            tc.tile_pool(name="persist", bufs=1) as pp,
            tc.tile_pool(name="work", bufs=3) as wp,
            tc.tile_pool(name="gather", bufs=1) as gp,
            tc.tile_pool(name="psum", bufs=2, space="PSUM") as psp,
            tc.tile_pool(name="psav", bufs=2, space="PSUM") as pav,
        ):
            tab_sb = cp.tile([128, TABLE], fp32)
            nc.sync.dma_start(tab_sb[:, :], tab_d)
            idx_sb = cp.tile([128, N], i16)
            nc.sync.dma_start(idx_sb[:, :], idx_d)
            wqk_sb = cp.tile([128, 4, 128], bf16)
            nc.sync.dma_start(wqk_sb[:, :, :], wqk_d.rearrange("(kc p) m -> p kc m", p=128))
            wv_sb = cp.tile([128, 4, 2 * D], bf16)
            nc.sync.dma_start(wv_sb[:, :, :], wv_d.rearrange("(kc p) m -> p kc m", p=128))
            wo_sb = cp.tile([128, 4, OUP], bf16)
            nc.sync.dma_start(wo_sb[:, :, :], wo_d.rearrange("(kc p) m -> p kc m", p=128))
            bo_sb = cp.tile([128, 4], fp32)
            nc.sync.dma_start(bo_sb[:, :], bo_d)
            e16_sb = cp.tile([16, 4, 128], bf16)
            nc.sync.dma_start(e16_sb[:, :, :], e16_d)

            b8full = cp.tile([128, 8, HPC, N], bias_dt)

            NIDX = 16 * N
            NCHUNK = nchunk
            CH = NIDX // NCHUNK

            def emit_frontend():
                for ch in range(NCHUNK):
                    gath = gp.tile([128, CH], fp32, tag="gath", bufs=1)
                    if skip_gather:
                        nc.gpsimd.memset(gath[:, :], 0.01)
                    else:
                        nc.gpsimd.ap_gather(
                            out_ap=gath[:, :],
                            in_ap=tab_sb[:, :],
                            idxs_ap=idx_sb[:, ch * (CH // 16):(ch + 1) * (CH // 16)],
                            channels=128,
                            num_elems=TABLE,
                            d=1,
                            num_idxs=CH,
                        )
                    b8 = gp.tile([128, CH], bias_dt, tag="b8", bufs=1)
                    nc.gpsimd.tensor_copy(b8[:, :], gath[:, :])
                    nc.gpsimd.dma_start(
                        ebc_in.rearrange("r g (ch f) -> g r ch f", f=CH)[:, :, ch, :],
                        b8[:, :],
                    )
                if collectives:
                    nc.gpsimd.collective_compute(
                        "AllToAll",
                        mybir.AluOpType.bypass,
                        replica_groups=RG,
                        ins=[ebc_in.opt()],
                        outs=[ebc_out.opt()],
                    )
                else:
                    nc.gpsimd.dma_start(
                        ebc_out.rearrange("s e g f -> (s e) g f"), ebc_in.opt())

            def emit_loads():
                for kb in range(8):
                    for e in range(HPC):
                        nc.gpsimd.dma_start(
                            b8full[:, kb, e, :],
                            ebc_out[kb, e].rearrange("g (fq q) -> (g fq) q", q=N),
                        )

            for _rep in range(repeat):
                c_sb = pp.tile([128, 8, HPC * N], bf16, name="c_sb")
                if skip_bias:
                    nc.vector.memset(c_sb[:, :, :], 1.0)
                else:
                    if _rep == 0:
                        emit_frontend()
                        emit_loads()
                    for kb in range(8):
                        for e in range(HPC):
                            nc.vector.tensor_scalar_add(
                                c_sb[:, kb, e * N:(e + 1) * N], b8full[:, kb, e, :], 1.0,
                            )
                q_sb = pp.tile([64, B, N], bf16, name="q_sb")
                k_sb = pp.tile([64, B, N], bf16, name="k_sb")
                v_sb = pp.tile([128, B, 8, HPC, D + 1], bf16, name="v_sb")
                nc.vector.memset(v_sb[:, :, :, :, D], 1.0)

                for b in range(B):
                    x_t = wp.tile([128, 4, N], bf16, tag="xt", bufs=2)
                    nc.sync.dma_start(x_t[:, :, :], x_d[b].rearrange("(kc p) n -> p kc n", p=128))

                    qk_ps = psp.tile([128, N], fp32, tag="st", bufs=stb,
                                     padded_shape=stpad)
                    for kc in range(4):
                        for half in range(2):
                            sl = slice(512 * half, 512 * (half + 1))
                            nc.tensor.matmul(
                                qk_ps[:, sl],
                                wqk_sb[:, kc, :],
                                x_t[:, kc, sl],
                                start=(kc == 0),
                                stop=(kc == 3),
                            )
                    nc.vector.tensor_copy(q_sb[:, b, :], qk_ps[0:64, :])
                    nc.vector.tensor_copy(k_sb[:, b, :], qk_ps[64:128, :])

                    for tbq in range(2):
                        v_ps = psp.tile([128, 4, 2 * D], fp32, tag="st", bufs=stb,
                                        padded_shape=[128, 4, (N if split_st else 2 * N) // 4])
                        for tb4 in range(4):
                            tb = 4 * tbq + tb4
                            for kc in range(4):
                                nc.tensor.matmul(
                                    v_ps[:, tb4, :],
                                    x_t[:, kc, 128 * tb:128 * (tb + 1)],
                                    wv_sb[:, kc, :],
                                    start=(kc == 0),
                                    stop=(kc == 3),
                                )
                        nc.vector.tensor_copy(
                            v_sb[:, b, 4 * tbq:4 * (tbq + 1), :, 0:D],
                            v_ps[:, :, :].rearrange("p t (h d) -> p t h d", h=HPC),
                        )

                attnout_sb = pp.tile([128, B, N], bf16, name="attnout_sb")

                for b in range(B):
                    avh = [pav.tile([D + 1, N], fp32, tag=f"av{h}", bufs=1,
                                    padded_shape=[128, N], name=f"av{h}")
                           for h in range(HPC)]
                    for kb in range(8):
                        if split_st:
                            for h in range(HPC):
                                st = psp.tile([128, N], fp32, tag="st", bufs=stb,
                                              padded_shape=stpad)
                                for half in range(2):
                                    sl = slice(512 * half, 512 * (half + 1))
                                    nc.tensor.matmul(
                                        st[:, sl],
                                        k_sb[32 * h:32 * (h + 1), b, 128 * kb:128 * (kb + 1)],
                                        q_sb[32 * h:32 * (h + 1), b, sl],
                                        start=True,
                                        stop=True,
                                    )
                                at = wp.tile([128, N], bf16, tag="attn", bufs=at_bufs)
                                nc.scalar.activation(at[:, :], st[:, :], mybir.ActivationFunctionType.Exp)
                                if not skip_mult:
                                    nc.vector.tensor_mul(at[:, :], at[:, :],
                                                         c_sb[:, kb, h * N:(h + 1) * N])
                                for half in range(2):
                                    sl = slice(512 * half, 512 * (half + 1))
                                    nc.tensor.matmul(
                                        avh[h][:, sl],
                                        v_sb[:, b, kb, h, :],
                                        at[:, sl],
                                        start=(kb == 0),
                                        stop=(kb == 7),
                                    )
                            continue
                        st2 = psp.tile([128, 2 * N], fp32, tag="st", bufs=stb)
                        for h in range(HPC):
                            for half in range(2):
                                sl = slice(512 * half, 512 * (half + 1))
                                nc.tensor.matmul(
                                    st2[:, h * N + 512 * half:h * N + 512 * (half + 1)],
                                    k_sb[32 * h:32 * (h + 1), b, 128 * kb:128 * (kb + 1)],
                                    q_sb[32 * h:32 * (h + 1), b, sl],
                                    start=True,
                                    stop=True,
                                )
                        at = wp.tile([128, 2 * N], bf16, tag="attn", bufs=at_bufs)
                        nc.scalar.activation(at[:, :], st2[:, :], mybir.ActivationFunctionType.Exp)
                        if not skip_mult:
                            nc.vector.tensor_mul(at[:, :], at[:, :], c_sb[:, kb, :])
                        for h in range(HPC):
                            for half in range(2):
                                sl = slice(512 * half, 512 * (half + 1))
                                nc.tensor.matmul(
                                    avh[h][:, sl],
                                    v_sb[:, b, kb, h, :],
                                    at[:, h * N + 512 * half:h * N + 512 * (half + 1)],
                                    start=(kb == 0),
                                    stop=(kb == 7),
                                )
                    for h in range(HPC):
                        nc.vector.tensor_copy(
                            attnout_sb[64 * h:64 * h + D + 1, b, :],
                            avh[h][:, :],
                        )
                    nc.sync.dma_start(ao_in[b][0:D + 1], attnout_sb[0:D + 1, b, :])
                    nc.sync.dma_start(ao_in[b][D + 1:], attnout_sb[64:64 + D + 1, b, :])

                if not skip_bias and _rep + 1 < repeat:
                    emit_frontend()

                if collectives:
                    nc.gpsimd.collective_compute(
                        "AllToAll",
                        mybir.AluOpType.bypass,
                        replica_groups=RG,
                        ins=[ao_in.opt()],
                        outs=[ao_out.opt()],
                    )
                else:
                    nc.gpsimd.dma_start(ao_out.opt(), ao_in.opt())
                if not skip_bias and _rep + 1 < repeat:
                    emit_loads()

                den_sb = wp.tile([16, N], bf16, tag="den", bufs=1)
                nc.sync.dma_start(
                    den_sb[:, :],
                    ao_out.rearrange("s (h x) n -> (s h) x n", x=D + 1)[:, D, :],
                )
                rec_sb = wp.tile([16, N], bf16, tag="rec", bufs=1)
                with nc.allow_low_precision(reason="bf16 softmax denominators are within tolerance"):
                    nc.vector.reciprocal(rec_sb[:, :], den_sb[:, :])

                ao_sb = pp.tile([128, 4, N], bf16, name="ao_sb")
                for kc in range(4):
                    for j in range(2):
                        for h in range(HPC):
                            nc.sync.dma_start(
                                ao_sb[64 * j + 32 * h:64 * j + 32 * h + D, kc, :],
                                ao_out[2 * kc + j, (D + 1) * h:(D + 1) * h + D, :],
                            )
                for kc in range(4):
                    bc_ps = pav.tile([128, N], fp32, tag="av0", bufs=1)
                    for half in range(2):
                        sl = slice(512 * half, 512 * (half + 1))
                        nc.tensor.matmul(bc_ps[:, sl], e16_sb[:, kc, :], rec_sb[:, sl],
                                         start=True, stop=True)
                    nc.vector.tensor_mul(ao_sb[:, kc, :], ao_sb[:, kc, :], bc_ps[:, :])

                for mb in range(4):
                    o_ps = psp.tile([128, N], fp32, tag="st", bufs=stb,
                                    padded_shape=stpad)
                    for kc in range(4):
                        for half in range(2):
                            sl = slice(512 * half, 512 * (half + 1))
                            nc.tensor.matmul(
                                o_ps[:, sl],
                                wo_sb[:, kc, 128 * mb:128 * (mb + 1)],
                                ao_sb[:, kc, sl],
                                start=(kc == 0),
                                stop=(kc == 3),
                            )
                    o_sb = wp.tile([128, N], fp32, tag="osb", bufs=2)
                    nc.vector.tensor_scalar_add(o_sb[:, :], o_ps[:, :], bo_sb[:, mb:mb + 1])
                    nc.sync.dma_start(out_d[128 * mb:128 * (mb + 1), :], o_sb[:, :])

    nc.compile()
    return nc


# ============================================================================
# host-side prep
# ============================================================================

def _common_prep(inputs):
    x = np.asarray(inputs["x"], np.float32).reshape(B, C, N)
    w_qkv = np.asarray(inputs["w_qkv"], np.float32)
    w_out = np.asarray(inputs["w_out"], np.float32)
    b_out = np.asarray(inputs["b_out"], np.float32)

    x_bf = np.ascontiguousarray(x.astype(BF))
    wq = w_qkv[0:OUP]
    wk = w_qkv[OUP:2 * OUP]
    wv = w_qkv[2 * OUP:3 * OUP]
    w_outT = np.ascontiguousarray(w_out.T.astype(BF))
    b_out4 = np.ascontiguousarray(b_out.reshape(4, 128).T.astype(np.float32))
    e16_bf = np.ascontiguousarray(E16.astype(BF))

    per_core = []
    for c in range(NCORES):
        h0, h1 = 2 * c, 2 * c + 1
        wqk_cols = np.concatenate(
            [
                wq[h0 * D:(h0 + 1) * D] * SCALE,
                wq[h1 * D:(h1 + 1) * D] * SCALE,
                wk[h0 * D:(h0 + 1) * D],
                wk[h1 * D:(h1 + 1) * D],
            ],
            axis=0,
        )
        w_qkT = np.ascontiguousarray(wqk_cols.T.astype(BF))
        wv_cols = np.concatenate(
            [wv[h0 * D:(h0 + 1) * D], wv[h1 * D:(h1 + 1) * D]], axis=0
        )
        w_vT = np.ascontiguousarray(wv_cols.T.astype(BF))
        per_core.append({
            "x": x_bf,
            "w_qkT": w_qkT,
            "w_vT": w_vT,
            "w_outT": w_outT,
            "b_out4": b_out4,
            "e16": e16_bf,
        })
    return per_core


def _prepare_in_maps(inputs):
    """v4 fast path: per-core exp'd, window-expanded bias tables."""
    table = np.asarray(inputs["bias_table"], np.float32)
    c_table = np.exp(table.astype(np.float64)).astype(np.float32)  # [3969, 16]
    # window expansion: idx[kx, u, qx] = 63*u + qx + 31 - kx  (all in range)
    kx = np.arange(32)[:, None, None]
    u = np.arange(63)[None, :, None]
    qx = np.arange(32)[None, None, :]
    widx = (63 * u + qx + 31 - kx).reshape(-1)  # [32*63*32]
    in_maps = _common_prep(inputs)
    for c in range(NCORES):
        be = np.stack([c_table[widx, 2 * c], c_table[widx, 2 * c + 1]])
        be = np.tile(be[:, None, :], (1, 4, 1)).reshape(2, 258048)
        in_maps[c]["bias_e"] = np.ascontiguousarray(be.astype(BF))
    return in_maps


def _prepare_in_maps_gather(inputs):
    """v3 fallback: raw table + i16 gather indices."""
    table = np.asarray(inputs["bias_table"], np.float32)
    ridx = np.asarray(inputs["rel_index"]).astype(np.int64).reshape(N, N)
    tab_rep = np.ascontiguousarray(table.T[np.arange(128) % HEADS].astype(np.float32))
    in_maps = _common_prep(inputs)
    for c in range(NCORES):
        sl = ridx[:, KCH * c:KCH * (c + 1)]
        idxw = np.empty((128, N), np.int16)
        for g in range(8):
            arr = sl[:, 16 * g:16 * (g + 1)].T.reshape(-1)
            idxw[16 * g:16 * (g + 1)] = arr.reshape(N, 16).T
        in_maps[c]["table"] = tab_rep
        in_maps[c]["idx"] = np.ascontiguousarray(idxw)
    return in_maps


def _get_graph(**kw):
    key = ("v4",) + tuple(sorted(kw.items()))
    if key not in _GRAPH_CACHE:
        _GRAPH_CACHE[key] = _build_graph(**kw)
    return _GRAPH_CACHE[key]


def _get_graph_gather(**kw):
    key = ("v3",) + tuple(sorted(kw.items()))
    if key not in _GRAPH_CACHE:
        _GRAPH_CACHE[key] = _build_graph_gather(**kw)
    return _GRAPH_CACHE[key]


def run_on_hw(inputs, trace=False, **kw):
    from concourse.bass_utils import run_bass_kernel_spmd

    nc = _get_graph()
    in_maps = _prepare_in_maps(inputs)
    return run_bass_kernel_spmd(nc, in_maps, core_ids=list(range(NCORES)), trace=trace, **kw)


def run_on_hw_gather(inputs, trace=False, **kw):
    from concourse.bass_utils import run_bass_kernel_spmd

    nc = _get_graph_gather()
    in_maps = _prepare_in_maps_gather(inputs)
    return run_bass_kernel_spmd(nc, in_maps, core_ids=list(range(NCORES)), trace=trace, **kw)


def kernel(**inputs) -> np.ndarray:
    ridx = np.asarray(inputs["rel_index"]).ravel()
    if ridx.shape == (N * N,) and np.array_equal(ridx, _toeplitz_rel_index()):
        res = run_on_hw(inputs).results
    else:
        res = run_on_hw_gather(inputs).results
    out = np.stack([np.asarray(res[c]["out"], np.float32) for c in range(NCORES)])
    return out.reshape(B, OUP, 32, 32)


if __name__ == "__main__":
    _get_graph()
    print("graph built + compiled OK")
